# revision 45
# baseline (speedup 1.0000x reference)
"""Trainium2 Bass kernel for nn_Memory scatter_memory problem.

Reference computation:
    scale = t/(t+1) if t > 1 else 1
    inv   = 1/(t+1)
    entity_memory = entity_memory*scale ;  .at[nodes_ids].add((nodes_emb @ W_node.T + b_node)*inv)
    rel_memory    = rel_memory*scale    ;  .at[rels_ids].add((rels_emb @ W_rel.T + b_rel)*inv)
    out = concat([entity_memory, rel_memory])   # [100500, 512]

Strategy (8 NeuronCores, SPMD single program):
  - Row-shard entity_memory (12544 rows/core) and rel_memory (64 rows/core).
  - HOST routes each event to its owner core (by id range), sorts by local row id,
    pads to a common chunk count. Bias (zero in practice) is folded into the
    memory shards as cnt*inv*b/scale. Memory flows as bf16 (tolerance 2e-2).
  - NODES (~0.65 events/row): fp8e4 DoubleRow projection matmuls (K=256/pass,
    2x PE throughput), Act-engine scaled psum->SBUF copy, then scatter-add via
    one-hot f32r matmuls into per-row-group PSUM tiles.
  - RELS (~131 events/row; fp8 would lose sqrt(131)x precision): aggregate-first.
    One-hot fp16 matmuls (lhsT=emb k-slice, rhs=onehot[ev,64]) accumulate
    S.T = sum of embeddings per rel row in PSUM [128k, 8, 64]; project S once
    at the end through fp16 W_rel*inv. Exact up to fp16.
  - Host reassembles the full [100500, 512] f32 output from per-core shards.
"""

import os
import sys
import numpy as np

for _p in ("/root/.axon_site", "/root/.axon_site/_ro/trn_rl_repo",
           "/root/.axon_site/_ro/pypackages", "/opt/trn_rl_repo"):
    if os.path.isdir(_p) and _p not in sys.path:
        sys.path.append(_p)

import ml_dtypes
import concourse.bacc as bacc
import concourse.mybir as mybir
import concourse.tile as tile
from concourse.bass_utils import run_bass_kernel_spmd

F32 = mybir.dt.float32
F32R = mybir.dt.float32r
F16 = mybir.dt.float16
BF16 = mybir.dt.bfloat16
FP8 = mybir.dt.float8e4
NP8 = ml_dtypes.float8_e4m3
NPBF = ml_dtypes.bfloat16
AL = mybir.AluOpType
DR = mybir.MatmulPerfMode.DoubleRow

N_NODES = 100000
N_RELS = 500
MEM_DIM = 512
IN_DIM = 1024
NCORES = 8
NSHARD = 12544          # 98 * 128 rows per core (core 7 ragged, padded)
NGROUPS = NSHARD // 128  # 98
RSHARD = 64             # rel rows per core (core 7 ragged, padded)
KT = IN_DIM // 128      # 8 k-tiles
NJ = KT // 2            # 4 DoubleRow pairs
W_PRESCALE = 96.0       # host folds inv*this into W_node so fp8 stays normal
PAD_ID = 1.0e6

_module_cache = {}


def _id_groups(last_pair):
    """Groups whose merge runs via the PE+Act path (host pre-scales their mem
    rows by `scale`). Empty: the identity seed stalled the in-order PE queue
    on the mem DMA and measured slower than the pure-DVE merge path."""
    return set()


def _ensure_ntff_hook():
    """Register the axon NTFF profile hook (missing antenv.axon_hooks shim)."""
    import types
    try:
        from antenv.axon_hooks import get_axon_ntff_profile_hook
        return get_axon_ntff_profile_hook() is not None
    except ImportError:
        pass
    try:
        import antenv
        from trn_agent_boot.trn_boot import _ntff_profile_via_ctypes
        import concourse.bass_utils as bu
        mod = types.ModuleType("antenv.axon_hooks")
        state = {"h": None}
        mod.set_axon_ntff_profile_hook = lambda h: state.__setitem__("h", h)
        mod.get_axon_ntff_profile_hook = lambda: state["h"]
        sys.modules["antenv.axon_hooks"] = mod
        antenv.axon_hooks = mod
        h = _ntff_profile_via_ctypes("/opt/axon/libaxon_pjrt.so")
        mod.set_axon_ntff_profile_hook(h)
        bu.upload_artifacts = lambda tmpdir: f"local:{tmpdir}"
        return h is not None
    except Exception:
        return False


def _build_module(NCn, NCr, spans_n):
    """Build the SPMD Bacc module.

    NCn/NCr: number of 128-event chunks for nodes/rels.
    spans_n: list over ec of sorted group lists (union over cores).
    """
    nc = bacc.Bacc(None, target_bir_lowering=False)

    NPn = (NCn + 1) // 2
    # nodes per chunk: [p=128 (k%128), pair j=4, i=2, event=128] fp8
    emb_n = nc.dram_tensor("emb_n", [NPn, 128, 2 * IN_DIM], FP8, kind="ExternalInput")
    # rels per chunk: [p=128 (event), k=1024] fp16
    emb_r = nc.dram_tensor("emb_r", [NCr, 128, IN_DIM], F16, kind="ExternalInput")
    ids_n = nc.dram_tensor("ids_n", [128, NCn], F32, kind="ExternalInput")
    ids_r = nc.dram_tensor("ids_r", [128, NCr], F32, kind="ExternalInput")
    w_n = nc.dram_tensor("w_n", [128, NJ * 2 * MEM_DIM], FP8, kind="ExternalInput")
    w_r = nc.dram_tensor("w_r", [128, KT * MEM_DIM], F16, kind="ExternalInput")
    s_col = nc.dram_tensor("s_col", [128, 1], F32, kind="ExternalInput")
    iota_in = nc.dram_tensor("iota_in", [128, 128], F32, kind="ExternalInput")
    iota16_in = nc.dram_tensor("iota16_in", [128, RSHARD], F16, kind="ExternalInput")
    ident8_in = nc.dram_tensor("ident8_in", [128, 128], FP8, kind="ExternalInput")
    mem = nc.dram_tensor("mem", [NSHARD, MEM_DIM], FP8, kind="ExternalInput")
    rmem = nc.dram_tensor("rmem", [RSHARD, MEM_DIM], FP8, kind="ExternalInput")
    out_n = nc.dram_tensor("out_n", [NSHARD, MEM_DIM], BF16, kind="ExternalOutput")
    out_r = nc.dram_tensor("out_r", [RSHARD, MEM_DIM], BF16, kind="ExternalOutput")

    # scatter runs per chunk-PAIR (fp8 DoubleRow: K=256 = 2 event chunks)
    NPAIR = (NCn + 1) // 2
    pair_chunks = [[c for c in (2 * a, 2 * a + 1) if c < NCn]
                   for a in range(NPAIR)]
    spans_pair = [sorted(set().union(*[spans_n[c] for c in pcs]))
                  for pcs in pair_chunks]
    touch_in_pair = {}   # (a, g) -> list of pair-local chunk slots (0/1)
    for a, pcs in enumerate(pair_chunks):
        for ec in pcs:
            for g in spans_n[ec]:
                touch_in_pair.setdefault((a, g), []).append(ec % 2)
    last_pair = {}
    for a, gs in enumerate(spans_pair):
        for g in gs:
            last_pair[g] = a
    merge_after = [[] for _ in range(NPAIR)]
    for g, a in last_pair.items():
        merge_after[a].append(g)
    untouched = [g for g in range(NGROUPS) if g not in last_pair]

    # PSUM budget: proj double-buffer + open scatter groups + rel agg/proj banks
    maxopen = 0
    open_now = set()
    for a, gs in enumerate(spans_pair):
        open_now.update(gs)
        maxopen = max(maxopen, len(open_now))
        for g in merge_after[a]:
            open_now.discard(g)
    pu_bufs = 2 if maxopen <= 5 else 1
    pg_bufs = min(max(maxopen, 1), 8 - pu_bufs - 1)

    # ~40% of merges take the PE+Act path (psum seeded with identity@mem, so
    # the merge is a plain Act copy) to unload the saturated DVE
    id_groups = _id_groups(last_pair)

    with tile.TileContext(nc) as tc:
        with tc.tile_pool(name="const", bufs=1) as cpool, \
             tc.tile_pool(name="stage", bufs=6) as spool, \
             tc.tile_pool(name="rstage", bufs=8) as rspool, \
             tc.tile_pool(name="work", bufs=14) as wpool, \
             tc.tile_pool(name="oh", bufs=16) as ohpool, \
             tc.tile_pool(name="updp", bufs=8) as updpool, \
             tc.tile_pool(name="pu", bufs=pu_bufs, space="PSUM") as pupool, \
             tc.tile_pool(name="pg", bufs=pg_bufs, space="PSUM") as pgpool, \
             tc.tile_pool(name="pr", bufs=1, space="PSUM") as prpool:

            # ---- constants (W first: PE-critical path) ----
            t_wn = cpool.tile([128, NJ, 2, MEM_DIM], FP8, tag="wn")
            nc.sync.dma_start(t_wn[:], w_n.ap().rearrange(
                "p (j i n) -> p j i n", j=NJ, i=2))
            t_iota = cpool.tile([128, 128], F32, tag="iota")
            nc.scalar.dma_start(t_iota[:], iota_in[:])
            t_iota16 = cpool.tile([128, RSHARD], F16, tag="iota16")
            nc.scalar.dma_start(t_iota16[:], iota16_in[:])
            t_ids_n = cpool.tile([128, NCn], F32, tag="idsn")
            nc.scalar.dma_start(t_ids_n[:], ids_n[:])
            t_ids_r = cpool.tile([128, NCr], F32, tag="idsr")
            nc.scalar.dma_start(t_ids_r[:], ids_r[:])
            t_s = cpool.tile([128, 1], F32, tag="scol")
            nc.scalar.dma_start(t_s[:], s_col[:])
            t_id8 = cpool.tile([128, 128], FP8, tag="id8")
            nc.scalar.dma_start(t_id8[:], ident8_in[:])

            def fetch_mem(g):
                t_mem = wpool.tile([128, MEM_DIM], FP8, tag="memst",
                                   name=f"mem_{g}")
                nc.sync.dma_start(t_mem[:], mem[g * 128:(g + 1) * 128, :])
                grp_mem[g] = t_mem

            def merge_group(g):
                t_out = wpool.tile([128, MEM_DIM], BF16, tag="outsb")
                if g in id_groups:
                    # mem (host pre-scaled) was seeded into the psum by an
                    # identity matmul at group open; merge is a plain Act copy
                    nc.scalar.copy(t_out[:], grp_psum[g][:])
                    del grp_psum[g]
                    nc.gpsimd.dma_start(out_n[g * 128:(g + 1) * 128, :], t_out[:])
                    return
                if g not in grp_mem:
                    fetch_mem(g)
                t_mem = grp_mem.pop(g)
                if g in grp_psum:
                    # merges otherwise on DVE: GPSIMD cannot access PSUM
                    nc.vector.scalar_tensor_tensor(
                        t_out[:], t_mem[:], t_s[:, 0:1], grp_psum[g][:],
                        op0=AL.mult, op1=AL.add)
                    del grp_psum[g]
                else:
                    nc.vector.tensor_scalar_mul(t_out[:], t_mem[:], t_s[:, 0:1])
                # out-DMA issues on the (otherwise idle) gpsimd queue: on sync
                # or scalar it would head-of-line block independent work
                nc.gpsimd.dma_start(out_n[g * 128:(g + 1) * 128, :], t_out[:])

            grp_psum = {}
            grp_mem = {}
            oh_pair = {}
            upd_pair = {}
            pair_n = {}
            upd_scale = 1.0 / W_PRESCALE
            PFP = 3   # node emb prefetch distance (pairs of chunks)
            PFR = 5   # rel emb prefetch distance (chunks)

            def issue_emb_n(pi):
                if pi * 2 < NCn and pi not in pair_n:
                    t_pp = spool.tile([128, 2, NJ, 2, 128], FP8, tag="er",
                                      name=f"en_{pi}")
                    nc.sync.dma_start(
                        t_pp[:], emb_n[pi].rearrange(
                            "p (c j i e) -> p c j i e", c=2, j=NJ, i=2))
                    pair_n[pi] = t_pp

            def node_proj(ec):
                issue_emb_n(ec // 2 + PFP)
                a, half = ec // 2, ec % 2
                t_en = pair_n[a][:, half]
                p_u = pupool.tile([128, MEM_DIM], F32, tag="pu", name=f"pun_{ec}")
                for j in range(NJ):
                    nc.tensor.matmul(p_u[:], t_en[:, j], t_wn[:, j],
                                     start=(j == 0), stop=(j == NJ - 1),
                                     perf_mode=DR)
                for g in spans_n[ec]:
                    if (a, g) not in oh_pair:
                        oh_pair[(a, g)] = ohpool.tile(
                            [128, 2, 128], FP8, tag="oh", name=f"oh_{a}_{g}")
                    # one-hot builds stay on DVE: gpsimd's software
                    # tensor_scalar measured ~20x slower than modeled
                    nc.vector.tensor_scalar(
                        oh_pair[(a, g)][:, half], t_iota[:], float(g * 128),
                        t_ids_n[:, ec:ec + 1], op0=AL.add, op1=AL.is_equal)
                if half == 0:
                    upd_pair[a] = updpool.tile([128, 2, MEM_DIM], FP8, tag="upd",
                                               name=f"updn_{a}")
                nc.scalar.mul(upd_pair[a][:, half], p_u[:], upd_scale)

            def node_scatter_pair(a):
                t_updp = upd_pair.pop(a)
                for g in spans_pair[a]:
                    if g not in grp_psum:
                        grp_psum[g] = pgpool.tile([128, MEM_DIM], F32, tag="pg",
                                                  name=f"pg_{g}")
                        fetch_mem(g)  # prefetch the memory rows for the merge
                        first = True
                    else:
                        first = False
                    t_ohp = oh_pair.pop((a, g))
                    stop = last_pair[g] == a
                    slots = touch_in_pair[(a, g)]
                    if len(slots) == 2:
                        nc.tensor.matmul(grp_psum[g][:], t_ohp[:], t_updp[:],
                                         start=first, stop=stop, perf_mode=DR,
                                         skip_group_check=True)
                    else:
                        i = slots[0]
                        nc.tensor.matmul(grp_psum[g][:], t_ohp[:, i], t_updp[:, i],
                                         start=first, stop=stop,
                                         skip_group_check=True)
                for g in sorted(merge_after[a]):
                    merge_group(g)

            rel_er = {}

            def issue_emb_r(ec):
                if ec < NCr and ec not in rel_er:
                    t_er = rspool.tile([128, KT, 128], F16, tag="err",
                                       name=f"er_{ec}")
                    nc.sync.dma_start(
                        t_er[:], emb_r[ec].rearrange("p (k e) -> p k e", k=KT))
                    rel_er[ec] = t_er

            def rel_agg(ec):
                """Accumulate S.T[k, rel_row] += emb[ev, k].T @ onehot[ev, row]."""
                issue_emb_r(ec + PFR)
                t_er = rel_er.pop(ec)
                t_oh = ohpool.tile([128, RSHARD], F16, tag="ohr", name=f"ohr_{ec}")
                nc.vector.tensor_scalar(
                    t_oh[:], t_iota16[:], 0.0, t_ids_r[:, ec:ec + 1],
                    op0=AL.add, op1=AL.is_equal)
                # start=False always: a per-k-slice start would zero the WHOLE
                # tile, wiping sibling slices; the tile is memset once instead.
                for k in range(KT):
                    nc.tensor.matmul(p_aggT[:, k, :], t_er[:, k, :], t_oh[:],
                                     start=False, stop=(ec == NCr - 1),
                                     skip_group_check=True)

            # rel aggregation PSUM: S.T layout [128 (k%128), kt=8, 64 rel rows]
            # (shares its bank with p_rel below — lifetimes are sequential)
            p_aggT = prpool.tile([128, KT, RSHARD], F32, tag="pr", name="pagg")
            nc.vector.memset(p_aggT[:], 0.0)

            # prime the embedding prefetch pipelines
            for pi in range(PFP):
                issue_emb_n(pi)
            for ci in range(PFR):
                issue_emb_r(ci)

            # software-pipelined emission: scatter runs one chunk-pair behind
            # proj, node/rel interleaved to smooth the engine mix
            steps = []
            for i in range(max(NCn, NCr)):
                if i < NCn:
                    steps.append(("n", i))
                if i < NCr:
                    steps.append(("r", i))
            LAGP = 1
            for kind, i in steps:
                if kind == "n":
                    node_proj(i)
                    a = i // 2
                    if (i % 2 == 1 or i == NCn - 1) and a >= LAGP:
                        node_scatter_pair(a - LAGP)
                else:
                    rel_agg(i)
            for a in range(max(NPAIR - LAGP, 0), NPAIR):
                node_scatter_pair(a)

            for g in untouched:
                merge_group(g)

            # ---- rel projection: S.T -> SBUF fp16, then 8 matmuls ----
            # (w_r loads here, deferred out of the head; the sync queue runs
            # far ahead of compute so it still lands before the matmuls)
            t_wr = cpool.tile([128, KT, MEM_DIM], F16, tag="wr")
            nc.sync.dma_start(t_wr[:], w_r.ap().rearrange("p (k n) -> p k n", k=KT))
            t_sT = wpool.tile([128, KT, RSHARD], F16, tag="sT")
            nc.scalar.copy(t_sT[:], p_aggT[:])
            p_rel = prpool.tile([128, MEM_DIM], F32, tag="pr", name="prel")
            for k in range(KT):
                nc.tensor.matmul(p_rel[:RSHARD, :], t_sT[:, k, :], t_wr[:, k, :],
                                 start=(k == 0), stop=(k == KT - 1))

            # ---- rel merge ----
            t_rmem = wpool.tile([128, MEM_DIM], FP8, tag="memst")
            nc.sync.dma_start(t_rmem[:RSHARD, :], rmem[:])
            t_rout = wpool.tile([128, MEM_DIM], BF16, tag="outsb")
            nc.vector.scalar_tensor_tensor(
                t_rout[:RSHARD, :], t_rmem[:RSHARD, :], t_s[:RSHARD, 0:1],
                p_rel[:RSHARD, :], op0=AL.mult, op1=AL.add)
            nc.gpsimd.dma_start(out_r[:], t_rout[:RSHARD, :])

    nc.finalize()
    return nc


def _route(ids, n_rows_per_core):
    """Route events to owner cores; sort by local id.

    Returns (perm[core] event indices sorted by local id, NC common chunk count).
    """
    owner = np.minimum(ids // n_rows_per_core, NCORES - 1)
    perms = []
    for c in range(NCORES):
        ev = np.nonzero(owner == c)[0]
        loc = ids[ev] - c * n_rows_per_core
        order = np.argsort(loc, kind="stable")
        perms.append(ev[order])
    nmax = max(len(p) for p in perms)
    NC = (nmax + 127) // 128
    return perms, max(NC, 1)


def _pack_emb_nodes(embT_q, perm, NC):
    """embT_q [IN_DIM, B] fp8 -> [NP, 128, 2*IN_DIM] DoubleRow pair layout.

    Per chunk, partition p holds [j, i, e] with k = (2j+i)*128 + p.
    """
    n = len(perm)
    C = NC * 128
    g = np.zeros((IN_DIM, C), dtype=embT_q.dtype)
    g[:, :n] = embT_q[:, perm]
    # [ (j i p), (c e) ] -> [c, p, j, i, e]
    g = g.reshape(NJ, 2, 128, NC, 128).transpose(3, 2, 0, 1, 4)
    g = g.reshape(NC, 128, IN_DIM)
    NP = (NC + 1) // 2
    if NP * 2 != NC:
        g = np.concatenate([g, np.zeros((1, 128, IN_DIM), g.dtype)], axis=0)
    g = g.reshape(NP, 2, 128, IN_DIM).transpose(0, 2, 1, 3).reshape(NP, 128, 2 * IN_DIM)
    return np.ascontiguousarray(g)


def _pack_emb_rels(emb_q, perm, NC):
    """emb_q [B, IN_DIM] fp16 -> [NC, 128, IN_DIM], partition = event."""
    n = len(perm)
    C = NC * 128
    g = np.zeros((C, IN_DIM), dtype=emb_q.dtype)
    g[:n] = emb_q[perm]
    return np.ascontiguousarray(g.reshape(NC, 128, IN_DIM))


def _pack_w_nodes(W, inv):
    """W [MEM_DIM, IN_DIM] -> fp8(W.T * inv * W_PRESCALE) in [128, j, i, m]."""
    wq = (W.T * np.float32(inv * W_PRESCALE)).astype(NP8)  # [IN_DIM, MEM_DIM]
    wq = wq.reshape(NJ, 2, 128, MEM_DIM).transpose(2, 0, 1, 3)
    return np.ascontiguousarray(wq.reshape(128, NJ * 2 * MEM_DIM))


def _pack_w_rels(W, inv):
    """W [MEM_DIM, IN_DIM] -> fp16(W.T * inv) in [128, k, m]."""
    wq = (W.T * np.float32(inv)).astype(np.float16)
    wq = wq.reshape(KT, 128, MEM_DIM).transpose(1, 0, 2)
    return np.ascontiguousarray(wq.reshape(128, KT * MEM_DIM))


def _pack_ids(local_ids, NC):
    n = len(local_ids)
    C = NC * 128
    out = np.full(C, PAD_ID, dtype=np.float32)
    out[:n] = local_ids.astype(np.float32)
    return np.ascontiguousarray(out.reshape(NC, 128).T)  # [128, NC]


def _spans(local_sorted_per_core, NC):
    spans = [set() for _ in range(NC)]
    for loc in local_sorted_per_core:
        for ec in range(NC):
            seg = loc[ec * 128:(ec + 1) * 128]
            if len(seg) == 0:
                continue
            for g in range(int(seg[0]) // 128, int(seg[-1]) // 128 + 1):
                spans[ec].add(g)
    return [sorted(s) for s in spans]


def kernel(nodes_embeddings, rels_embeddings, nodes_ids, rels_ids,
           entity_memory, rel_memory, W_node, b_node, W_rel, b_rel, time):
    nodes_embeddings = np.ascontiguousarray(np.asarray(nodes_embeddings, dtype=np.float32))
    rels_embeddings = np.ascontiguousarray(np.asarray(rels_embeddings, dtype=np.float32))
    nodes_ids = np.asarray(nodes_ids).astype(np.int64)
    rels_ids = np.asarray(rels_ids).astype(np.int64)
    entity_memory = np.asarray(entity_memory, dtype=np.float32)
    rel_memory = np.asarray(rel_memory, dtype=np.float32)
    W_node = np.asarray(W_node, dtype=np.float32)
    b_node = np.asarray(b_node, dtype=np.float32)
    W_rel = np.asarray(W_rel, dtype=np.float32)
    b_rel = np.asarray(b_rel, dtype=np.float32)
    t = float(np.asarray(time))

    inv = np.float32(1.0 / (t + 1.0))
    scale = np.float32(t / (t + 1.0)) if t > 1 else np.float32(1.0)

    # ---- host routing ----
    perms_n, NCn = _route(nodes_ids, NSHARD)
    perms_r, NCr = _route(rels_ids, RSHARD)

    loc_n = [nodes_ids[p] - c * NSHARD for c, p in enumerate(perms_n)]
    spans_n = _spans(loc_n, NCn)

    # mirror the module's merge-path split (host pre-scales id-group mem rows)
    NPAIR = (NCn + 1) // 2
    last_pair = {}
    for a in range(NPAIR):
        for ec in (2 * a, 2 * a + 1):
            if ec < NCn:
                for g in spans_n[ec]:
                    last_pair[g] = a
    idg = sorted(_id_groups(last_pair))

    key = (NCn, NCr, tuple(tuple(s) for s in spans_n))
    if key not in _module_cache:
        _module_cache[key] = _build_module(NCn, NCr, spans_n)
    nc = _module_cache[key]

    # ---- host packing ----
    embT_n = nodes_embeddings.astype(NP8).T  # [IN_DIM, B] fp8
    emb_r16 = rels_embeddings.astype(np.float16)  # [B, IN_DIM] fp16
    wn = _pack_w_nodes(W_node, inv)
    wr = _pack_w_rels(W_rel, inv)
    s_col = np.full((128, 1), scale, dtype=np.float32)
    iota = np.broadcast_to(np.arange(128, dtype=np.float32), (128, 128)).copy()
    iota16 = np.broadcast_to(np.arange(RSHARD, dtype=np.float16), (128, RSHARD)).copy()
    ident8 = np.eye(128, dtype=np.float32).astype(NP8)

    in_maps = []
    for c in range(NCORES):
        lo_n, hi_n = c * NSHARD, min((c + 1) * NSHARD, N_NODES)
        lo_r, hi_r = c * RSHARD, min((c + 1) * RSHARD, N_RELS)
        mem_shard = np.zeros((NSHARD, MEM_DIM), dtype=np.float32)
        mem_shard[:hi_n - lo_n] = entity_memory[lo_n:hi_n]
        rmem_shard = np.zeros((RSHARD, MEM_DIM), dtype=np.float32)
        rmem_shard[:hi_r - lo_r] = rel_memory[lo_r:hi_r]
        loc_r = rels_ids[perms_r[c]] - c * RSHARD
        # fold bias: device computes out = mem*scale + sum(updates); each event
        # contributes inv*b less than the reference, so pre-add cnt*inv*b/scale.
        if b_node.any():
            cnt = np.bincount(loc_n[c], minlength=NSHARD).astype(np.float32)
            mem_shard += (cnt[:, None] * (inv / scale)) * b_node[None, :]
        if b_rel.any():
            cntr = np.bincount(loc_r, minlength=RSHARD).astype(np.float32)
            rmem_shard += (cntr[:, None] * (inv / scale)) * b_rel[None, :]
        for g in idg:
            mem_shard[g * 128:(g + 1) * 128] *= scale
        in_maps.append(dict(
            emb_n=_pack_emb_nodes(embT_n, perms_n[c], NCn),
            emb_r=_pack_emb_rels(emb_r16, perms_r[c], NCr),
            ids_n=_pack_ids(loc_n[c], NCn),
            ids_r=_pack_ids(loc_r, NCr),
            w_n=wn, w_r=wr, s_col=s_col, iota_in=iota, iota16_in=iota16,
            ident8_in=ident8, mem=mem_shard.astype(NP8),
            rmem=rmem_shard.astype(NP8),
        ))

    trace = bool(int(os.environ.get("KERNEL_TRACE", "0"))) and _ensure_ntff_hook()
    try:
        res = run_bass_kernel_spmd(
            nc, in_maps, core_ids=list(range(NCORES)),
            trace=trace, trace_cores=list(range(NCORES)) if trace else None)
    except Exception:
        # transient device faults (e.g. NRT_EXEC_UNIT_UNRECOVERABLE) recover
        # on re-dispatch; retry once
        res = run_bass_kernel_spmd(
            nc, in_maps, core_ids=list(range(NCORES)),
            trace=trace, trace_cores=list(range(NCORES)) if trace else None)
    kernel.last_exec_time_ns = res.exec_time_ns
    kernel.last_results = res

    out = np.empty((N_NODES + N_RELS, MEM_DIM), dtype=np.float32)
    for c in range(NCORES):
        lo_n, hi_n = c * NSHARD, min((c + 1) * NSHARD, N_NODES)
        out[lo_n:hi_n] = res.results[c]["out_n"][:hi_n - lo_n].astype(np.float32)
        lo_r, hi_r = c * RSHARD, min((c + 1) * RSHARD, N_RELS)
        out[N_NODES + lo_r:N_NODES + hi_r] = \
            res.results[c]["out_r"][:hi_r - lo_r].astype(np.float32)
    return out


# revision 46
# speedup vs baseline: 1.0675x; 1.0675x over previous
"""Trainium2 Bass kernel for nn_Memory scatter_memory problem.

Reference computation:
    scale = t/(t+1) if t > 1 else 1
    inv   = 1/(t+1)
    entity_memory = entity_memory*scale ;  .at[nodes_ids].add((nodes_emb @ W_node.T + b_node)*inv)
    rel_memory    = rel_memory*scale    ;  .at[rels_ids].add((rels_emb @ W_rel.T + b_rel)*inv)
    out = concat([entity_memory, rel_memory])   # [100500, 512]

Strategy (8 NeuronCores, SPMD single program):
  - Row-shard entity_memory (12544 rows/core) and rel_memory (64 rows/core).
  - HOST routes each event to its owner core (by id range), sorts by local row id,
    pads to a common chunk count. Bias (zero in practice) is folded into the
    memory shards as cnt*inv*b/scale. Memory flows as bf16 (tolerance 2e-2).
  - NODES (~0.65 events/row): fp8e4 DoubleRow projection matmuls (K=256/pass,
    2x PE throughput), Act-engine scaled psum->SBUF copy, then scatter-add via
    one-hot f32r matmuls into per-row-group PSUM tiles.
  - RELS (~131 events/row; fp8 would lose sqrt(131)x precision): aggregate-first.
    One-hot fp16 matmuls (lhsT=emb k-slice, rhs=onehot[ev,64]) accumulate
    S.T = sum of embeddings per rel row in PSUM [128k, 8, 64]; project S once
    at the end through fp16 W_rel*inv. Exact up to fp16.
  - Host reassembles the full [100500, 512] f32 output from per-core shards.
"""

import os
import sys
import numpy as np

for _p in ("/root/.axon_site", "/root/.axon_site/_ro/trn_rl_repo",
           "/root/.axon_site/_ro/pypackages", "/opt/trn_rl_repo"):
    if os.path.isdir(_p) and _p not in sys.path:
        sys.path.append(_p)

import ml_dtypes
import concourse.bacc as bacc
import concourse.mybir as mybir
import concourse.tile as tile
from concourse.bass_utils import run_bass_kernel_spmd

F32 = mybir.dt.float32
F32R = mybir.dt.float32r
F16 = mybir.dt.float16
BF16 = mybir.dt.bfloat16
FP8 = mybir.dt.float8e4
NP8 = ml_dtypes.float8_e4m3
NPBF = ml_dtypes.bfloat16
AL = mybir.AluOpType
DR = mybir.MatmulPerfMode.DoubleRow

N_NODES = 100000
N_RELS = 500
MEM_DIM = 512
IN_DIM = 1024
NCORES = 8
NSHARD = 12544          # 98 * 128 rows per core (core 7 ragged, padded)
NGROUPS = NSHARD // 128  # 98
RSHARD = 64             # rel rows per core (core 7 ragged, padded)
KT = IN_DIM // 128      # 8 k-tiles
NJ = KT // 2            # 4 DoubleRow pairs
W_PRESCALE = 96.0       # host folds inv*this into W_node so fp8 stays normal
PAD_ID = 1.0e6

_module_cache = {}


def _id_groups(last_pair):
    """Groups whose merge runs via the PE+Act path (host pre-scales their mem
    rows by `scale`). Empty: the identity seed stalled the in-order PE queue
    on the mem DMA and measured slower than the pure-DVE merge path."""
    return set()


def _ensure_ntff_hook():
    """Register the axon NTFF profile hook (missing antenv.axon_hooks shim)."""
    import types
    try:
        from antenv.axon_hooks import get_axon_ntff_profile_hook
        return get_axon_ntff_profile_hook() is not None
    except ImportError:
        pass
    try:
        import antenv
        from trn_agent_boot.trn_boot import _ntff_profile_via_ctypes
        import concourse.bass_utils as bu
        mod = types.ModuleType("antenv.axon_hooks")
        state = {"h": None}
        mod.set_axon_ntff_profile_hook = lambda h: state.__setitem__("h", h)
        mod.get_axon_ntff_profile_hook = lambda: state["h"]
        sys.modules["antenv.axon_hooks"] = mod
        antenv.axon_hooks = mod
        h = _ntff_profile_via_ctypes("/opt/axon/libaxon_pjrt.so")
        mod.set_axon_ntff_profile_hook(h)
        bu.upload_artifacts = lambda tmpdir: f"local:{tmpdir}"
        return h is not None
    except Exception:
        return False


def _build_module(NCn, NCr, spans_n):
    """Build the SPMD Bacc module.

    NCn/NCr: number of 128-event chunks for nodes/rels.
    spans_n: list over ec of sorted group lists (union over cores).
    """
    nc = bacc.Bacc(None, target_bir_lowering=False)

    NPn = (NCn + 1) // 2
    # nodes per chunk: [p=128 (k%128), pair j=4, i=2, event=128] fp8
    emb_n = nc.dram_tensor("emb_n", [NPn, 128, 2 * IN_DIM], FP8, kind="ExternalInput")
    # rels per chunk: [p=128 (event), k=1024] fp16
    emb_r = nc.dram_tensor("emb_r", [NCr, 128, IN_DIM], F16, kind="ExternalInput")
    ids_n = nc.dram_tensor("ids_n", [128, NCn], F32, kind="ExternalInput")
    ids_r = nc.dram_tensor("ids_r", [128, NCr], F32, kind="ExternalInput")
    w_n = nc.dram_tensor("w_n", [128, NJ * 2 * MEM_DIM], FP8, kind="ExternalInput")
    w_r = nc.dram_tensor("w_r", [128, KT * MEM_DIM], F16, kind="ExternalInput")
    s_col = nc.dram_tensor("s_col", [128, 1], F32, kind="ExternalInput")
    iota_in = nc.dram_tensor("iota_in", [128, 128], F32, kind="ExternalInput")
    iota16_in = nc.dram_tensor("iota16_in", [128, RSHARD], F16, kind="ExternalInput")
    ident8_in = nc.dram_tensor("ident8_in", [128, 128], FP8, kind="ExternalInput")
    mem = nc.dram_tensor("mem", [NSHARD, MEM_DIM], FP8, kind="ExternalInput")
    rmem = nc.dram_tensor("rmem", [RSHARD, MEM_DIM], FP8, kind="ExternalInput")
    out_n = nc.dram_tensor("out_n", [NSHARD, MEM_DIM], BF16, kind="ExternalOutput")
    out_r = nc.dram_tensor("out_r", [RSHARD, MEM_DIM], BF16, kind="ExternalOutput")

    # scatter runs per chunk-PAIR (fp8 DoubleRow: K=256 = 2 event chunks)
    NPAIR = (NCn + 1) // 2
    pair_chunks = [[c for c in (2 * a, 2 * a + 1) if c < NCn]
                   for a in range(NPAIR)]
    spans_pair = [sorted(set().union(*[spans_n[c] for c in pcs]))
                  for pcs in pair_chunks]
    touch_in_pair = {}   # (a, g) -> list of pair-local chunk slots (0/1)
    for a, pcs in enumerate(pair_chunks):
        for ec in pcs:
            for g in spans_n[ec]:
                touch_in_pair.setdefault((a, g), []).append(ec % 2)
    last_pair = {}
    for a, gs in enumerate(spans_pair):
        for g in gs:
            last_pair[g] = a
    merge_after = [[] for _ in range(NPAIR)]
    for g, a in last_pair.items():
        merge_after[a].append(g)
    untouched = [g for g in range(NGROUPS) if g not in last_pair]

    # PSUM budget: proj double-buffer + open scatter groups + rel agg/proj banks
    maxopen = 0
    open_now = set()
    for a, gs in enumerate(spans_pair):
        open_now.update(gs)
        maxopen = max(maxopen, len(open_now))
        for g in merge_after[a]:
            open_now.discard(g)
    pu_bufs = 2 if maxopen <= 5 else 1
    pg_bufs = min(max(maxopen, 1), 8 - pu_bufs - 1)

    # ~40% of merges take the PE+Act path (psum seeded with identity@mem, so
    # the merge is a plain Act copy) to unload the saturated DVE
    id_groups = _id_groups(last_pair)

    with tile.TileContext(nc) as tc:
        with tc.tile_pool(name="const", bufs=1) as cpool, \
             tc.tile_pool(name="stage", bufs=6) as spool, \
             tc.tile_pool(name="rstage", bufs=8) as rspool, \
             tc.tile_pool(name="work", bufs=14) as wpool, \
             tc.tile_pool(name="oh", bufs=16) as ohpool, \
             tc.tile_pool(name="updp", bufs=8) as updpool, \
             tc.tile_pool(name="pu", bufs=pu_bufs, space="PSUM") as pupool, \
             tc.tile_pool(name="pg", bufs=pg_bufs, space="PSUM") as pgpool, \
             tc.tile_pool(name="pr", bufs=1, space="PSUM") as prpool:

            # ---- constants (W first: PE-critical path) ----
            t_wn = cpool.tile([128, NJ, 2, MEM_DIM], FP8, tag="wn")
            nc.sync.dma_start(t_wn[:], w_n.ap().rearrange(
                "p (j i n) -> p j i n", j=NJ, i=2))
            t_wr = cpool.tile([128, KT, MEM_DIM], F16, tag="wr")
            nc.sync.dma_start(t_wr[:], w_r.ap().rearrange("p (k n) -> p k n", k=KT))
            t_iota = cpool.tile([128, 128], F32, tag="iota")
            nc.scalar.dma_start(t_iota[:], iota_in[:])
            t_iota16 = cpool.tile([128, RSHARD], F16, tag="iota16")
            nc.scalar.dma_start(t_iota16[:], iota16_in[:])
            t_ids_n = cpool.tile([128, NCn], F32, tag="idsn")
            nc.scalar.dma_start(t_ids_n[:], ids_n[:])
            t_ids_r = cpool.tile([128, NCr], F32, tag="idsr")
            nc.scalar.dma_start(t_ids_r[:], ids_r[:])
            t_s = cpool.tile([128, 1], F32, tag="scol")
            nc.scalar.dma_start(t_s[:], s_col[:])
            t_id8 = cpool.tile([128, 128], FP8, tag="id8")
            nc.scalar.dma_start(t_id8[:], ident8_in[:])

            def fetch_mem(g):
                t_mem = wpool.tile([128, MEM_DIM], FP8, tag="memst",
                                   name=f"mem_{g}")
                nc.sync.dma_start(t_mem[:], mem[g * 128:(g + 1) * 128, :])
                grp_mem[g] = t_mem

            def merge_group(g):
                t_out = wpool.tile([128, MEM_DIM], BF16, tag="outsb")
                if g in id_groups:
                    # mem (host pre-scaled) was seeded into the psum by an
                    # identity matmul at group open; merge is a plain Act copy
                    nc.scalar.copy(t_out[:], grp_psum[g][:])
                    del grp_psum[g]
                    nc.gpsimd.dma_start(out_n[g * 128:(g + 1) * 128, :], t_out[:])
                    return
                if g not in grp_mem:
                    fetch_mem(g)
                t_mem = grp_mem.pop(g)
                if g in grp_psum:
                    # merges otherwise on DVE: GPSIMD cannot access PSUM
                    nc.vector.scalar_tensor_tensor(
                        t_out[:], t_mem[:], t_s[:, 0:1], grp_psum[g][:],
                        op0=AL.mult, op1=AL.add)
                    del grp_psum[g]
                else:
                    nc.vector.tensor_scalar_mul(t_out[:], t_mem[:], t_s[:, 0:1])
                # out-DMA issues on the (otherwise idle) gpsimd queue: on sync
                # or scalar it would head-of-line block independent work
                nc.gpsimd.dma_start(out_n[g * 128:(g + 1) * 128, :], t_out[:])

            grp_psum = {}
            grp_mem = {}
            oh_pair = {}
            upd_pair = {}
            pair_n = {}
            upd_scale = 1.0 / W_PRESCALE
            PFP = 3   # node emb prefetch distance (pairs of chunks)
            PFR = 5   # rel emb prefetch distance (chunks)

            def issue_emb_n(pi):
                if pi * 2 < NCn and pi not in pair_n:
                    t_pp = spool.tile([128, 2, NJ, 2, 128], FP8, tag="er",
                                      name=f"en_{pi}")
                    nc.sync.dma_start(
                        t_pp[:], emb_n[pi].rearrange(
                            "p (c j i e) -> p c j i e", c=2, j=NJ, i=2))
                    pair_n[pi] = t_pp

            def node_proj(ec):
                issue_emb_n(ec // 2 + PFP)
                a, half = ec // 2, ec % 2
                t_en = pair_n[a][:, half]
                p_u = pupool.tile([128, MEM_DIM], F32, tag="pu", name=f"pun_{ec}")
                for j in range(NJ):
                    nc.tensor.matmul(p_u[:], t_en[:, j], t_wn[:, j],
                                     start=(j == 0), stop=(j == NJ - 1),
                                     perf_mode=DR)
                for g in spans_n[ec]:
                    if (a, g) not in oh_pair:
                        oh_pair[(a, g)] = ohpool.tile(
                            [128, 2, 128], FP8, tag="oh", name=f"oh_{a}_{g}")
                    # one-hot builds stay on DVE: gpsimd's software
                    # tensor_scalar measured ~20x slower than modeled
                    nc.vector.tensor_scalar(
                        oh_pair[(a, g)][:, half], t_iota[:], float(g * 128),
                        t_ids_n[:, ec:ec + 1], op0=AL.add, op1=AL.is_equal)
                if half == 0:
                    upd_pair[a] = updpool.tile([128, 2, MEM_DIM], FP8, tag="upd",
                                               name=f"updn_{a}")
                nc.scalar.mul(upd_pair[a][:, half], p_u[:], upd_scale)

            def node_scatter_pair(a):
                t_updp = upd_pair.pop(a)
                for g in spans_pair[a]:
                    if g not in grp_psum:
                        grp_psum[g] = pgpool.tile([128, MEM_DIM], F32, tag="pg",
                                                  name=f"pg_{g}")
                        fetch_mem(g)  # prefetch the memory rows for the merge
                        first = True
                    else:
                        first = False
                    t_ohp = oh_pair.pop((a, g))
                    stop = last_pair[g] == a
                    slots = touch_in_pair[(a, g)]
                    if len(slots) == 2:
                        nc.tensor.matmul(grp_psum[g][:], t_ohp[:], t_updp[:],
                                         start=first, stop=stop, perf_mode=DR,
                                         skip_group_check=True)
                    else:
                        i = slots[0]
                        nc.tensor.matmul(grp_psum[g][:], t_ohp[:, i], t_updp[:, i],
                                         start=first, stop=stop,
                                         skip_group_check=True)
                for g in sorted(merge_after[a]):
                    merge_group(g)

            rel_er = {}

            def issue_emb_r(ec):
                if ec < NCr and ec not in rel_er:
                    t_er = rspool.tile([128, KT, 128], F16, tag="err",
                                       name=f"er_{ec}")
                    nc.sync.dma_start(
                        t_er[:], emb_r[ec].rearrange("p (k e) -> p k e", k=KT))
                    rel_er[ec] = t_er

            def rel_agg(ec):
                """Accumulate S.T[k, rel_row] += emb[ev, k].T @ onehot[ev, row]."""
                issue_emb_r(ec + PFR)
                t_er = rel_er.pop(ec)
                t_oh = ohpool.tile([128, RSHARD], F16, tag="ohr", name=f"ohr_{ec}")
                nc.vector.tensor_scalar(
                    t_oh[:], t_iota16[:], 0.0, t_ids_r[:, ec:ec + 1],
                    op0=AL.add, op1=AL.is_equal)
                # start=False always: a per-k-slice start would zero the WHOLE
                # tile, wiping sibling slices; the tile is memset once instead.
                for k in range(KT):
                    nc.tensor.matmul(p_aggT[:, k, :], t_er[:, k, :], t_oh[:],
                                     start=False, stop=(ec == NCr - 1),
                                     skip_group_check=True)

            # rel aggregation PSUM: S.T layout [128 (k%128), kt=8, 64 rel rows]
            # (shares its bank with p_rel below — lifetimes are sequential)
            p_aggT = prpool.tile([128, KT, RSHARD], F32, tag="pr", name="pagg")
            nc.vector.memset(p_aggT[:], 0.0)

            # prime the embedding prefetch pipelines
            for pi in range(PFP):
                issue_emb_n(pi)
            for ci in range(PFR):
                issue_emb_r(ci)

            # software-pipelined emission: scatter runs one chunk-pair behind
            # proj, node/rel interleaved to smooth the engine mix
            steps = []
            for i in range(max(NCn, NCr)):
                if i < NCn:
                    steps.append(("n", i))
                if i < NCr:
                    steps.append(("r", i))
            LAGP = 1
            for kind, i in steps:
                if kind == "n":
                    node_proj(i)
                    a = i // 2
                    if (i % 2 == 1 or i == NCn - 1) and a >= LAGP:
                        node_scatter_pair(a - LAGP)
                else:
                    rel_agg(i)
            for a in range(max(NPAIR - LAGP, 0), NPAIR):
                node_scatter_pair(a)

            for g in untouched:
                merge_group(g)

            # ---- rel projection: S.T -> SBUF fp16, then 8 matmuls ----
            t_sT = wpool.tile([128, KT, RSHARD], F16, tag="sT")
            nc.scalar.copy(t_sT[:], p_aggT[:])
            p_rel = prpool.tile([128, MEM_DIM], F32, tag="pr", name="prel")
            for k in range(KT):
                nc.tensor.matmul(p_rel[:RSHARD, :], t_sT[:, k, :], t_wr[:, k, :],
                                 start=(k == 0), stop=(k == KT - 1))

            # ---- rel merge ----
            t_rmem = wpool.tile([128, MEM_DIM], FP8, tag="memst")
            nc.sync.dma_start(t_rmem[:RSHARD, :], rmem[:])
            t_rout = wpool.tile([128, MEM_DIM], BF16, tag="outsb")
            nc.vector.scalar_tensor_tensor(
                t_rout[:RSHARD, :], t_rmem[:RSHARD, :], t_s[:RSHARD, 0:1],
                p_rel[:RSHARD, :], op0=AL.mult, op1=AL.add)
            nc.gpsimd.dma_start(out_r[:], t_rout[:RSHARD, :])

    nc.finalize()
    return nc


def _route(ids, n_rows_per_core):
    """Route events to owner cores; sort by local id.

    Returns (perm[core] event indices sorted by local id, NC common chunk count).
    """
    owner = np.minimum(ids // n_rows_per_core, NCORES - 1)
    perms = []
    for c in range(NCORES):
        ev = np.nonzero(owner == c)[0]
        loc = ids[ev] - c * n_rows_per_core
        order = np.argsort(loc, kind="stable")
        perms.append(ev[order])
    nmax = max(len(p) for p in perms)
    NC = (nmax + 127) // 128
    return perms, max(NC, 1)


def _pack_emb_nodes(embT_q, perm, NC):
    """embT_q [IN_DIM, B] fp8 -> [NP, 128, 2*IN_DIM] DoubleRow pair layout.

    Per chunk, partition p holds [j, i, e] with k = (2j+i)*128 + p.
    """
    n = len(perm)
    C = NC * 128
    g = np.zeros((IN_DIM, C), dtype=embT_q.dtype)
    g[:, :n] = embT_q[:, perm]
    # [ (j i p), (c e) ] -> [c, p, j, i, e]
    g = g.reshape(NJ, 2, 128, NC, 128).transpose(3, 2, 0, 1, 4)
    g = g.reshape(NC, 128, IN_DIM)
    NP = (NC + 1) // 2
    if NP * 2 != NC:
        g = np.concatenate([g, np.zeros((1, 128, IN_DIM), g.dtype)], axis=0)
    g = g.reshape(NP, 2, 128, IN_DIM).transpose(0, 2, 1, 3).reshape(NP, 128, 2 * IN_DIM)
    return np.ascontiguousarray(g)


def _pack_emb_rels(emb_q, perm, NC):
    """emb_q [B, IN_DIM] fp16 -> [NC, 128, IN_DIM], partition = event."""
    n = len(perm)
    C = NC * 128
    g = np.zeros((C, IN_DIM), dtype=emb_q.dtype)
    g[:n] = emb_q[perm]
    return np.ascontiguousarray(g.reshape(NC, 128, IN_DIM))


def _pack_w_nodes(W, inv):
    """W [MEM_DIM, IN_DIM] -> fp8(W.T * inv * W_PRESCALE) in [128, j, i, m]."""
    wq = (W.T * np.float32(inv * W_PRESCALE)).astype(NP8)  # [IN_DIM, MEM_DIM]
    wq = wq.reshape(NJ, 2, 128, MEM_DIM).transpose(2, 0, 1, 3)
    return np.ascontiguousarray(wq.reshape(128, NJ * 2 * MEM_DIM))


def _pack_w_rels(W, inv):
    """W [MEM_DIM, IN_DIM] -> fp16(W.T * inv) in [128, k, m]."""
    wq = (W.T * np.float32(inv)).astype(np.float16)
    wq = wq.reshape(KT, 128, MEM_DIM).transpose(1, 0, 2)
    return np.ascontiguousarray(wq.reshape(128, KT * MEM_DIM))


def _pack_ids(local_ids, NC):
    n = len(local_ids)
    C = NC * 128
    out = np.full(C, PAD_ID, dtype=np.float32)
    out[:n] = local_ids.astype(np.float32)
    return np.ascontiguousarray(out.reshape(NC, 128).T)  # [128, NC]


def _spans(local_sorted_per_core, NC):
    spans = [set() for _ in range(NC)]
    for loc in local_sorted_per_core:
        for ec in range(NC):
            seg = loc[ec * 128:(ec + 1) * 128]
            if len(seg) == 0:
                continue
            for g in range(int(seg[0]) // 128, int(seg[-1]) // 128 + 1):
                spans[ec].add(g)
    return [sorted(s) for s in spans]


def kernel(nodes_embeddings, rels_embeddings, nodes_ids, rels_ids,
           entity_memory, rel_memory, W_node, b_node, W_rel, b_rel, time):
    nodes_embeddings = np.ascontiguousarray(np.asarray(nodes_embeddings, dtype=np.float32))
    rels_embeddings = np.ascontiguousarray(np.asarray(rels_embeddings, dtype=np.float32))
    nodes_ids = np.asarray(nodes_ids).astype(np.int64)
    rels_ids = np.asarray(rels_ids).astype(np.int64)
    entity_memory = np.asarray(entity_memory, dtype=np.float32)
    rel_memory = np.asarray(rel_memory, dtype=np.float32)
    W_node = np.asarray(W_node, dtype=np.float32)
    b_node = np.asarray(b_node, dtype=np.float32)
    W_rel = np.asarray(W_rel, dtype=np.float32)
    b_rel = np.asarray(b_rel, dtype=np.float32)
    t = float(np.asarray(time))

    inv = np.float32(1.0 / (t + 1.0))
    scale = np.float32(t / (t + 1.0)) if t > 1 else np.float32(1.0)

    # ---- host routing ----
    perms_n, NCn = _route(nodes_ids, NSHARD)
    perms_r, NCr = _route(rels_ids, RSHARD)

    loc_n = [nodes_ids[p] - c * NSHARD for c, p in enumerate(perms_n)]
    spans_n = _spans(loc_n, NCn)

    # mirror the module's merge-path split (host pre-scales id-group mem rows)
    NPAIR = (NCn + 1) // 2
    last_pair = {}
    for a in range(NPAIR):
        for ec in (2 * a, 2 * a + 1):
            if ec < NCn:
                for g in spans_n[ec]:
                    last_pair[g] = a
    idg = sorted(_id_groups(last_pair))

    key = (NCn, NCr, tuple(tuple(s) for s in spans_n))
    if key not in _module_cache:
        _module_cache[key] = _build_module(NCn, NCr, spans_n)
    nc = _module_cache[key]

    # ---- host packing ----
    embT_n = nodes_embeddings.astype(NP8).T  # [IN_DIM, B] fp8
    emb_r16 = rels_embeddings.astype(np.float16)  # [B, IN_DIM] fp16
    wn = _pack_w_nodes(W_node, inv)
    wr = _pack_w_rels(W_rel, inv)
    s_col = np.full((128, 1), scale, dtype=np.float32)
    iota = np.broadcast_to(np.arange(128, dtype=np.float32), (128, 128)).copy()
    iota16 = np.broadcast_to(np.arange(RSHARD, dtype=np.float16), (128, RSHARD)).copy()
    ident8 = np.eye(128, dtype=np.float32).astype(NP8)

    in_maps = []
    for c in range(NCORES):
        lo_n, hi_n = c * NSHARD, min((c + 1) * NSHARD, N_NODES)
        lo_r, hi_r = c * RSHARD, min((c + 1) * RSHARD, N_RELS)
        mem_shard = np.zeros((NSHARD, MEM_DIM), dtype=np.float32)
        mem_shard[:hi_n - lo_n] = entity_memory[lo_n:hi_n]
        rmem_shard = np.zeros((RSHARD, MEM_DIM), dtype=np.float32)
        rmem_shard[:hi_r - lo_r] = rel_memory[lo_r:hi_r]
        loc_r = rels_ids[perms_r[c]] - c * RSHARD
        # fold bias: device computes out = mem*scale + sum(updates); each event
        # contributes inv*b less than the reference, so pre-add cnt*inv*b/scale.
        if b_node.any():
            cnt = np.bincount(loc_n[c], minlength=NSHARD).astype(np.float32)
            mem_shard += (cnt[:, None] * (inv / scale)) * b_node[None, :]
        if b_rel.any():
            cntr = np.bincount(loc_r, minlength=RSHARD).astype(np.float32)
            rmem_shard += (cntr[:, None] * (inv / scale)) * b_rel[None, :]
        for g in idg:
            mem_shard[g * 128:(g + 1) * 128] *= scale
        in_maps.append(dict(
            emb_n=_pack_emb_nodes(embT_n, perms_n[c], NCn),
            emb_r=_pack_emb_rels(emb_r16, perms_r[c], NCr),
            ids_n=_pack_ids(loc_n[c], NCn),
            ids_r=_pack_ids(loc_r, NCr),
            w_n=wn, w_r=wr, s_col=s_col, iota_in=iota, iota16_in=iota16,
            ident8_in=ident8, mem=mem_shard.astype(NP8),
            rmem=rmem_shard.astype(NP8),
        ))

    trace = bool(int(os.environ.get("KERNEL_TRACE", "0"))) and _ensure_ntff_hook()
    try:
        res = run_bass_kernel_spmd(
            nc, in_maps, core_ids=list(range(NCORES)),
            trace=trace, trace_cores=list(range(NCORES)) if trace else None)
    except Exception:
        # transient device faults (e.g. NRT_EXEC_UNIT_UNRECOVERABLE) recover
        # on re-dispatch; retry once
        res = run_bass_kernel_spmd(
            nc, in_maps, core_ids=list(range(NCORES)),
            trace=trace, trace_cores=list(range(NCORES)) if trace else None)
    kernel.last_exec_time_ns = res.exec_time_ns
    kernel.last_results = res

    out = np.empty((N_NODES + N_RELS, MEM_DIM), dtype=np.float32)
    for c in range(NCORES):
        lo_n, hi_n = c * NSHARD, min((c + 1) * NSHARD, N_NODES)
        out[lo_n:hi_n] = res.results[c]["out_n"][:hi_n - lo_n].astype(np.float32)
        lo_r, hi_r = c * RSHARD, min((c + 1) * RSHARD, N_RELS)
        out[N_NODES + lo_r:N_NODES + hi_r] = \
            res.results[c]["out_r"][:hi_r - lo_r].astype(np.float32)
    return out


# revision 47
# speedup vs baseline: 1.1024x; 1.0327x over previous
"""Trainium2 Bass kernel for nn_Memory scatter_memory problem.

Reference computation:
    scale = t/(t+1) if t > 1 else 1
    inv   = 1/(t+1)
    entity_memory = entity_memory*scale ;  .at[nodes_ids].add((nodes_emb @ W_node.T + b_node)*inv)
    rel_memory    = rel_memory*scale    ;  .at[rels_ids].add((rels_emb @ W_rel.T + b_rel)*inv)
    out = concat([entity_memory, rel_memory])   # [100500, 512]

Strategy (8 NeuronCores, SPMD single program):
  - Row-shard entity_memory (12544 rows/core) and rel_memory (64 rows/core).
  - HOST routes each event to its owner core (by id range), sorts by local row id,
    pads to a common chunk count. Bias (zero in practice) is folded into the
    memory shards as cnt*inv*b/scale. Memory flows as bf16 (tolerance 2e-2).
  - NODES (~0.65 events/row): fp8e4 DoubleRow projection matmuls (K=256/pass,
    2x PE throughput), Act-engine scaled psum->SBUF copy, then scatter-add via
    one-hot f32r matmuls into per-row-group PSUM tiles.
  - RELS (~131 events/row; fp8 would lose sqrt(131)x precision): aggregate-first.
    One-hot fp16 matmuls (lhsT=emb k-slice, rhs=onehot[ev,64]) accumulate
    S.T = sum of embeddings per rel row in PSUM [128k, 8, 64]; project S once
    at the end through fp16 W_rel*inv. Exact up to fp16.
  - Host reassembles the full [100500, 512] f32 output from per-core shards.
"""

import os
import sys
import numpy as np

for _p in ("/root/.axon_site", "/root/.axon_site/_ro/trn_rl_repo",
           "/root/.axon_site/_ro/pypackages", "/opt/trn_rl_repo"):
    if os.path.isdir(_p) and _p not in sys.path:
        sys.path.append(_p)

import ml_dtypes
import concourse.bacc as bacc
import concourse.mybir as mybir
import concourse.tile as tile
from concourse.bass_utils import run_bass_kernel_spmd

F32 = mybir.dt.float32
F32R = mybir.dt.float32r
F16 = mybir.dt.float16
BF16 = mybir.dt.bfloat16
FP8 = mybir.dt.float8e4
NP8 = ml_dtypes.float8_e4m3
NPBF = ml_dtypes.bfloat16
AL = mybir.AluOpType
DR = mybir.MatmulPerfMode.DoubleRow

N_NODES = 100000
N_RELS = 500
MEM_DIM = 512
IN_DIM = 1024
NCORES = 8
NSHARD = 12544          # 98 * 128 rows per core (core 7 ragged, padded)
NGROUPS = NSHARD // 128  # 98
RSHARD = 64             # rel rows per core (core 7 ragged, padded)
KT = IN_DIM // 128      # 8 k-tiles
NJ = KT // 2            # 4 DoubleRow pairs
W_PRESCALE = 96.0       # host folds inv*this into W_node so fp8 stays normal
PAD_ID = 1.0e6

_module_cache = {}


def _id_groups(last_pair):
    """Groups whose merge runs via the PE+Act path (host pre-scales their mem
    rows by `scale`). Empty: the identity seed stalled the in-order PE queue
    on the mem DMA and measured slower than the pure-DVE merge path."""
    return set()


def _ensure_ntff_hook():
    """Register the axon NTFF profile hook (missing antenv.axon_hooks shim)."""
    import types
    try:
        from antenv.axon_hooks import get_axon_ntff_profile_hook
        return get_axon_ntff_profile_hook() is not None
    except ImportError:
        pass
    try:
        import antenv
        from trn_agent_boot.trn_boot import _ntff_profile_via_ctypes
        import concourse.bass_utils as bu
        mod = types.ModuleType("antenv.axon_hooks")
        state = {"h": None}
        mod.set_axon_ntff_profile_hook = lambda h: state.__setitem__("h", h)
        mod.get_axon_ntff_profile_hook = lambda: state["h"]
        sys.modules["antenv.axon_hooks"] = mod
        antenv.axon_hooks = mod
        h = _ntff_profile_via_ctypes("/opt/axon/libaxon_pjrt.so")
        mod.set_axon_ntff_profile_hook(h)
        bu.upload_artifacts = lambda tmpdir: f"local:{tmpdir}"
        return h is not None
    except Exception:
        return False


def _build_module(NCn, NCr, spans_n):
    """Build the SPMD Bacc module.

    NCn/NCr: number of 128-event chunks for nodes/rels.
    spans_n: list over ec of sorted group lists (union over cores).
    """
    nc = bacc.Bacc(None, target_bir_lowering=False)

    NPn = (NCn + 1) // 2
    # nodes per chunk: [p=128 (k%128), pair j=4, i=2, event=128] fp8
    emb_n = nc.dram_tensor("emb_n", [NPn, 128, 2 * IN_DIM], FP8, kind="ExternalInput")
    # rels per chunk: [p=128 (event), k=1024] fp16
    emb_r = nc.dram_tensor("emb_r", [NCr, 128, IN_DIM], F16, kind="ExternalInput")
    ids_n = nc.dram_tensor("ids_n", [128, NCn], F32, kind="ExternalInput")
    ids_r = nc.dram_tensor("ids_r", [128, NCr], F32, kind="ExternalInput")
    w_n = nc.dram_tensor("w_n", [128, NJ * 2 * MEM_DIM], FP8, kind="ExternalInput")
    w_r = nc.dram_tensor("w_r", [128, KT * MEM_DIM], F16, kind="ExternalInput")
    s_col = nc.dram_tensor("s_col", [128, 1], F32, kind="ExternalInput")
    iota_in = nc.dram_tensor("iota_in", [128, 128], F32, kind="ExternalInput")
    iota16_in = nc.dram_tensor("iota16_in", [128, RSHARD], F16, kind="ExternalInput")
    ident8_in = nc.dram_tensor("ident8_in", [128, 128], FP8, kind="ExternalInput")
    mem = nc.dram_tensor("mem", [NSHARD, MEM_DIM], FP8, kind="ExternalInput")
    rmem = nc.dram_tensor("rmem", [RSHARD, MEM_DIM], FP8, kind="ExternalInput")
    out_n = nc.dram_tensor("out_n", [NSHARD, MEM_DIM], BF16, kind="ExternalOutput")
    out_r = nc.dram_tensor("out_r", [RSHARD, MEM_DIM], BF16, kind="ExternalOutput")

    # scatter runs per chunk-PAIR (fp8 DoubleRow: K=256 = 2 event chunks)
    NPAIR = (NCn + 1) // 2
    pair_chunks = [[c for c in (2 * a, 2 * a + 1) if c < NCn]
                   for a in range(NPAIR)]
    spans_pair = [sorted(set().union(*[spans_n[c] for c in pcs]))
                  for pcs in pair_chunks]
    touch_in_pair = {}   # (a, g) -> list of pair-local chunk slots (0/1)
    for a, pcs in enumerate(pair_chunks):
        for ec in pcs:
            for g in spans_n[ec]:
                touch_in_pair.setdefault((a, g), []).append(ec % 2)
    last_pair = {}
    for a, gs in enumerate(spans_pair):
        for g in gs:
            last_pair[g] = a
    merge_after = [[] for _ in range(NPAIR)]
    for g, a in last_pair.items():
        merge_after[a].append(g)
    untouched = [g for g in range(NGROUPS) if g not in last_pair]

    # PSUM budget: proj double-buffer + open scatter groups + rel agg/proj banks
    maxopen = 0
    open_now = set()
    for a, gs in enumerate(spans_pair):
        open_now.update(gs)
        maxopen = max(maxopen, len(open_now))
        for g in merge_after[a]:
            open_now.discard(g)
    pu_bufs = 2 if maxopen <= 5 else 1
    pg_bufs = min(max(maxopen, 1), 8 - pu_bufs - 1)

    # ~40% of merges take the PE+Act path (psum seeded with identity@mem, so
    # the merge is a plain Act copy) to unload the saturated DVE
    id_groups = _id_groups(last_pair)

    with tile.TileContext(nc) as tc:
        with tc.tile_pool(name="const", bufs=1) as cpool, \
             tc.tile_pool(name="stage", bufs=6) as spool, \
             tc.tile_pool(name="rstage", bufs=8) as rspool, \
             tc.tile_pool(name="work", bufs=14) as wpool, \
             tc.tile_pool(name="oh", bufs=16) as ohpool, \
             tc.tile_pool(name="updp", bufs=8) as updpool, \
             tc.tile_pool(name="pu", bufs=pu_bufs, space="PSUM") as pupool, \
             tc.tile_pool(name="pg", bufs=pg_bufs, space="PSUM") as pgpool, \
             tc.tile_pool(name="pr", bufs=1, space="PSUM") as prpool:

            # ---- constants (W first: PE-critical path) ----
            t_wn = cpool.tile([128, NJ, 2, MEM_DIM], FP8, tag="wn")
            nc.sync.dma_start(t_wn[:], w_n.ap().rearrange(
                "p (j i n) -> p j i n", j=NJ, i=2))
            t_wr = cpool.tile([128, KT, MEM_DIM], F16, tag="wr")
            nc.sync.dma_start(t_wr[:], w_r.ap().rearrange("p (k n) -> p k n", k=KT))
            t_iota = cpool.tile([128, 128], F32, tag="iota")
            nc.scalar.dma_start(t_iota[:], iota_in[:])
            t_iota16 = cpool.tile([128, RSHARD], F16, tag="iota16")
            nc.scalar.dma_start(t_iota16[:], iota16_in[:])
            t_ids_n = cpool.tile([128, NCn], F32, tag="idsn")
            nc.scalar.dma_start(t_ids_n[:], ids_n[:])
            t_ids_r = cpool.tile([128, NCr], F32, tag="idsr")
            nc.scalar.dma_start(t_ids_r[:], ids_r[:])
            t_s = cpool.tile([128, 1], F32, tag="scol")
            nc.scalar.dma_start(t_s[:], s_col[:])
            t_id8 = cpool.tile([128, 128], FP8, tag="id8")
            nc.scalar.dma_start(t_id8[:], ident8_in[:])

            def fetch_mem(g):
                t_mem = wpool.tile([128, MEM_DIM], FP8, tag="memst",
                                   name=f"mem_{g}")
                nc.sync.dma_start(t_mem[:], mem[g * 128:(g + 1) * 128, :])
                grp_mem[g] = t_mem

            def _merge_into(g, t_out):
                if g not in grp_mem:
                    fetch_mem(g)
                t_mem = grp_mem.pop(g)
                if g in grp_psum:
                    # merges on DVE: GPSIMD cannot access PSUM
                    nc.vector.scalar_tensor_tensor(
                        t_out, t_mem[:], t_s[:, 0:1], grp_psum[g][:],
                        op0=AL.mult, op1=AL.add)
                    del grp_psum[g]
                else:
                    nc.vector.tensor_scalar_mul(t_out, t_mem[:], t_s[:, 0:1])

            def merge_groups(gs):
                """Merge groups gs; consecutive pairs share one out-DMA."""
                i = 0
                while i < len(gs):
                    if i + 1 < len(gs) and gs[i + 1] == gs[i] + 1:
                        pair = (gs[i], gs[i + 1])
                        t_out = wpool.tile([128, 2, MEM_DIM], BF16, tag="outsb")
                        _merge_into(pair[0], t_out[:, 0, :])
                        _merge_into(pair[1], t_out[:, 1, :])
                        # out-DMA on the idle gpsimd queue (SWDGE); one DMA
                        # covers both groups (256 consecutive rows)
                        nc.gpsimd.dma_start(
                            out_n[pair[0] * 128:(pair[0] + 2) * 128, :]
                            .rearrange("(t p) f -> p t f", t=2), t_out[:])
                        i += 2
                    else:
                        g = gs[i]
                        t_out = wpool.tile([128, 2, MEM_DIM], BF16, tag="outsb")
                        _merge_into(g, t_out[:, 0, :])
                        nc.gpsimd.dma_start(
                            out_n[g * 128:(g + 1) * 128, :], t_out[:, 0, :])
                        i += 1

            grp_psum = {}
            grp_mem = {}
            oh_pair = {}
            upd_pair = {}
            pair_n = {}
            upd_scale = 1.0 / W_PRESCALE
            PFP = 3   # node emb prefetch distance (pairs of chunks)
            PFR = 5   # rel emb prefetch distance (chunks)

            def issue_emb_n(pi):
                if pi * 2 < NCn and pi not in pair_n:
                    t_pp = spool.tile([128, 2, NJ, 2, 128], FP8, tag="er",
                                      name=f"en_{pi}")
                    nc.sync.dma_start(
                        t_pp[:], emb_n[pi].rearrange(
                            "p (c j i e) -> p c j i e", c=2, j=NJ, i=2))
                    pair_n[pi] = t_pp

            def node_proj(ec):
                issue_emb_n(ec // 2 + PFP)
                a, half = ec // 2, ec % 2
                t_en = pair_n[a][:, half]
                p_u = pupool.tile([128, MEM_DIM], F32, tag="pu", name=f"pun_{ec}")
                for j in range(NJ):
                    nc.tensor.matmul(p_u[:], t_en[:, j], t_wn[:, j],
                                     start=(j == 0), stop=(j == NJ - 1),
                                     perf_mode=DR)
                for g in spans_n[ec]:
                    if (a, g) not in oh_pair:
                        oh_pair[(a, g)] = ohpool.tile(
                            [128, 2, 128], FP8, tag="oh", name=f"oh_{a}_{g}")
                    # one-hot builds stay on DVE: gpsimd's software
                    # tensor_scalar measured ~20x slower than modeled
                    nc.vector.tensor_scalar(
                        oh_pair[(a, g)][:, half], t_iota[:], float(g * 128),
                        t_ids_n[:, ec:ec + 1], op0=AL.add, op1=AL.is_equal)
                if half == 0:
                    upd_pair[a] = updpool.tile([128, 2, MEM_DIM], FP8, tag="upd",
                                               name=f"updn_{a}")
                nc.scalar.mul(upd_pair[a][:, half], p_u[:], upd_scale)

            def node_scatter_pair(a):
                t_updp = upd_pair.pop(a)
                for g in spans_pair[a]:
                    if g not in grp_psum:
                        grp_psum[g] = pgpool.tile([128, MEM_DIM], F32, tag="pg",
                                                  name=f"pg_{g}")
                        fetch_mem(g)  # prefetch the memory rows for the merge
                        first = True
                    else:
                        first = False
                    t_ohp = oh_pair.pop((a, g))
                    stop = last_pair[g] == a
                    slots = touch_in_pair[(a, g)]
                    if len(slots) == 2:
                        nc.tensor.matmul(grp_psum[g][:], t_ohp[:], t_updp[:],
                                         start=first, stop=stop, perf_mode=DR,
                                         skip_group_check=True)
                    else:
                        i = slots[0]
                        nc.tensor.matmul(grp_psum[g][:], t_ohp[:, i], t_updp[:, i],
                                         start=first, stop=stop,
                                         skip_group_check=True)
                merge_groups(sorted(merge_after[a]))

            rel_er = {}

            def issue_emb_r(ec):
                if ec < NCr and ec not in rel_er:
                    t_er = rspool.tile([128, KT, 128], F16, tag="err",
                                       name=f"er_{ec}")
                    nc.sync.dma_start(
                        t_er[:], emb_r[ec].rearrange("p (k e) -> p k e", k=KT))
                    rel_er[ec] = t_er

            def rel_agg(ec):
                """Accumulate S.T[k, rel_row] += emb[ev, k].T @ onehot[ev, row]."""
                issue_emb_r(ec + PFR)
                t_er = rel_er.pop(ec)
                t_oh = ohpool.tile([128, RSHARD], F16, tag="ohr", name=f"ohr_{ec}")
                nc.vector.tensor_scalar(
                    t_oh[:], t_iota16[:], 0.0, t_ids_r[:, ec:ec + 1],
                    op0=AL.add, op1=AL.is_equal)
                # start=False always: a per-k-slice start would zero the WHOLE
                # tile, wiping sibling slices; the tile is memset once instead.
                for k in range(KT):
                    nc.tensor.matmul(p_aggT[:, k, :], t_er[:, k, :], t_oh[:],
                                     start=False, stop=(ec == NCr - 1),
                                     skip_group_check=True)

            # rel aggregation PSUM: S.T layout [128 (k%128), kt=8, 64 rel rows]
            # (shares its bank with p_rel below — lifetimes are sequential)
            p_aggT = prpool.tile([128, KT, RSHARD], F32, tag="pr", name="pagg")
            nc.vector.memset(p_aggT[:], 0.0)

            # prime the embedding prefetch pipelines
            for pi in range(PFP):
                issue_emb_n(pi)
            for ci in range(PFR):
                issue_emb_r(ci)

            # software-pipelined emission: scatter runs one chunk-pair behind
            # proj, node/rel interleaved to smooth the engine mix
            steps = []
            for i in range(max(NCn, NCr)):
                if i < NCn:
                    steps.append(("n", i))
                if i < NCr:
                    steps.append(("r", i))
            LAGP = 1
            for kind, i in steps:
                if kind == "n":
                    node_proj(i)
                    a = i // 2
                    if (i % 2 == 1 or i == NCn - 1) and a >= LAGP:
                        node_scatter_pair(a - LAGP)
                else:
                    rel_agg(i)
            for a in range(max(NPAIR - LAGP, 0), NPAIR):
                node_scatter_pair(a)

            merge_groups(sorted(untouched))

            # ---- rel projection: S.T -> SBUF fp16, then 8 matmuls ----
            t_sT = wpool.tile([128, KT, RSHARD], F16, tag="sT")
            nc.scalar.copy(t_sT[:], p_aggT[:])
            p_rel = prpool.tile([128, MEM_DIM], F32, tag="pr", name="prel")
            for k in range(KT):
                nc.tensor.matmul(p_rel[:RSHARD, :], t_sT[:, k, :], t_wr[:, k, :],
                                 start=(k == 0), stop=(k == KT - 1))

            # ---- rel merge ----
            t_rmem = wpool.tile([128, MEM_DIM], FP8, tag="memst")
            nc.sync.dma_start(t_rmem[:RSHARD, :], rmem[:])
            t_rout = wpool.tile([128, MEM_DIM], BF16, tag="outsb")
            nc.vector.scalar_tensor_tensor(
                t_rout[:RSHARD, :], t_rmem[:RSHARD, :], t_s[:RSHARD, 0:1],
                p_rel[:RSHARD, :], op0=AL.mult, op1=AL.add)
            nc.gpsimd.dma_start(out_r[:], t_rout[:RSHARD, :])

    nc.finalize()
    return nc


def _route(ids, n_rows_per_core):
    """Route events to owner cores; sort by local id.

    Returns (perm[core] event indices sorted by local id, NC common chunk count).
    """
    owner = np.minimum(ids // n_rows_per_core, NCORES - 1)
    perms = []
    for c in range(NCORES):
        ev = np.nonzero(owner == c)[0]
        loc = ids[ev] - c * n_rows_per_core
        order = np.argsort(loc, kind="stable")
        perms.append(ev[order])
    nmax = max(len(p) for p in perms)
    NC = (nmax + 127) // 128
    return perms, max(NC, 1)


def _pack_emb_nodes(embT_q, perm, NC):
    """embT_q [IN_DIM, B] fp8 -> [NP, 128, 2*IN_DIM] DoubleRow pair layout.

    Per chunk, partition p holds [j, i, e] with k = (2j+i)*128 + p.
    """
    n = len(perm)
    C = NC * 128
    g = np.zeros((IN_DIM, C), dtype=embT_q.dtype)
    g[:, :n] = embT_q[:, perm]
    # [ (j i p), (c e) ] -> [c, p, j, i, e]
    g = g.reshape(NJ, 2, 128, NC, 128).transpose(3, 2, 0, 1, 4)
    g = g.reshape(NC, 128, IN_DIM)
    NP = (NC + 1) // 2
    if NP * 2 != NC:
        g = np.concatenate([g, np.zeros((1, 128, IN_DIM), g.dtype)], axis=0)
    g = g.reshape(NP, 2, 128, IN_DIM).transpose(0, 2, 1, 3).reshape(NP, 128, 2 * IN_DIM)
    return np.ascontiguousarray(g)


def _pack_emb_rels(emb_q, perm, NC):
    """emb_q [B, IN_DIM] fp16 -> [NC, 128, IN_DIM], partition = event."""
    n = len(perm)
    C = NC * 128
    g = np.zeros((C, IN_DIM), dtype=emb_q.dtype)
    g[:n] = emb_q[perm]
    return np.ascontiguousarray(g.reshape(NC, 128, IN_DIM))


def _pack_w_nodes(W, inv):
    """W [MEM_DIM, IN_DIM] -> fp8(W.T * inv * W_PRESCALE) in [128, j, i, m]."""
    wq = (W.T * np.float32(inv * W_PRESCALE)).astype(NP8)  # [IN_DIM, MEM_DIM]
    wq = wq.reshape(NJ, 2, 128, MEM_DIM).transpose(2, 0, 1, 3)
    return np.ascontiguousarray(wq.reshape(128, NJ * 2 * MEM_DIM))


def _pack_w_rels(W, inv):
    """W [MEM_DIM, IN_DIM] -> fp16(W.T * inv) in [128, k, m]."""
    wq = (W.T * np.float32(inv)).astype(np.float16)
    wq = wq.reshape(KT, 128, MEM_DIM).transpose(1, 0, 2)
    return np.ascontiguousarray(wq.reshape(128, KT * MEM_DIM))


def _pack_ids(local_ids, NC):
    n = len(local_ids)
    C = NC * 128
    out = np.full(C, PAD_ID, dtype=np.float32)
    out[:n] = local_ids.astype(np.float32)
    return np.ascontiguousarray(out.reshape(NC, 128).T)  # [128, NC]


def _spans(local_sorted_per_core, NC):
    spans = [set() for _ in range(NC)]
    for loc in local_sorted_per_core:
        for ec in range(NC):
            seg = loc[ec * 128:(ec + 1) * 128]
            if len(seg) == 0:
                continue
            for g in range(int(seg[0]) // 128, int(seg[-1]) // 128 + 1):
                spans[ec].add(g)
    return [sorted(s) for s in spans]


def kernel(nodes_embeddings, rels_embeddings, nodes_ids, rels_ids,
           entity_memory, rel_memory, W_node, b_node, W_rel, b_rel, time):
    nodes_embeddings = np.ascontiguousarray(np.asarray(nodes_embeddings, dtype=np.float32))
    rels_embeddings = np.ascontiguousarray(np.asarray(rels_embeddings, dtype=np.float32))
    nodes_ids = np.asarray(nodes_ids).astype(np.int64)
    rels_ids = np.asarray(rels_ids).astype(np.int64)
    entity_memory = np.asarray(entity_memory, dtype=np.float32)
    rel_memory = np.asarray(rel_memory, dtype=np.float32)
    W_node = np.asarray(W_node, dtype=np.float32)
    b_node = np.asarray(b_node, dtype=np.float32)
    W_rel = np.asarray(W_rel, dtype=np.float32)
    b_rel = np.asarray(b_rel, dtype=np.float32)
    t = float(np.asarray(time))

    inv = np.float32(1.0 / (t + 1.0))
    scale = np.float32(t / (t + 1.0)) if t > 1 else np.float32(1.0)

    # ---- host routing ----
    perms_n, NCn = _route(nodes_ids, NSHARD)
    perms_r, NCr = _route(rels_ids, RSHARD)

    loc_n = [nodes_ids[p] - c * NSHARD for c, p in enumerate(perms_n)]
    spans_n = _spans(loc_n, NCn)

    # mirror the module's merge-path split (host pre-scales id-group mem rows)
    NPAIR = (NCn + 1) // 2
    last_pair = {}
    for a in range(NPAIR):
        for ec in (2 * a, 2 * a + 1):
            if ec < NCn:
                for g in spans_n[ec]:
                    last_pair[g] = a
    idg = sorted(_id_groups(last_pair))

    key = (NCn, NCr, tuple(tuple(s) for s in spans_n))
    if key not in _module_cache:
        _module_cache[key] = _build_module(NCn, NCr, spans_n)
    nc = _module_cache[key]

    # ---- host packing ----
    embT_n = nodes_embeddings.astype(NP8).T  # [IN_DIM, B] fp8
    emb_r16 = rels_embeddings.astype(np.float16)  # [B, IN_DIM] fp16
    wn = _pack_w_nodes(W_node, inv)
    wr = _pack_w_rels(W_rel, inv)
    s_col = np.full((128, 1), scale, dtype=np.float32)
    iota = np.broadcast_to(np.arange(128, dtype=np.float32), (128, 128)).copy()
    iota16 = np.broadcast_to(np.arange(RSHARD, dtype=np.float16), (128, RSHARD)).copy()
    ident8 = np.eye(128, dtype=np.float32).astype(NP8)

    in_maps = []
    for c in range(NCORES):
        lo_n, hi_n = c * NSHARD, min((c + 1) * NSHARD, N_NODES)
        lo_r, hi_r = c * RSHARD, min((c + 1) * RSHARD, N_RELS)
        mem_shard = np.zeros((NSHARD, MEM_DIM), dtype=np.float32)
        mem_shard[:hi_n - lo_n] = entity_memory[lo_n:hi_n]
        rmem_shard = np.zeros((RSHARD, MEM_DIM), dtype=np.float32)
        rmem_shard[:hi_r - lo_r] = rel_memory[lo_r:hi_r]
        loc_r = rels_ids[perms_r[c]] - c * RSHARD
        # fold bias: device computes out = mem*scale + sum(updates); each event
        # contributes inv*b less than the reference, so pre-add cnt*inv*b/scale.
        if b_node.any():
            cnt = np.bincount(loc_n[c], minlength=NSHARD).astype(np.float32)
            mem_shard += (cnt[:, None] * (inv / scale)) * b_node[None, :]
        if b_rel.any():
            cntr = np.bincount(loc_r, minlength=RSHARD).astype(np.float32)
            rmem_shard += (cntr[:, None] * (inv / scale)) * b_rel[None, :]
        for g in idg:
            mem_shard[g * 128:(g + 1) * 128] *= scale
        in_maps.append(dict(
            emb_n=_pack_emb_nodes(embT_n, perms_n[c], NCn),
            emb_r=_pack_emb_rels(emb_r16, perms_r[c], NCr),
            ids_n=_pack_ids(loc_n[c], NCn),
            ids_r=_pack_ids(loc_r, NCr),
            w_n=wn, w_r=wr, s_col=s_col, iota_in=iota, iota16_in=iota16,
            ident8_in=ident8, mem=mem_shard.astype(NP8),
            rmem=rmem_shard.astype(NP8),
        ))

    trace = bool(int(os.environ.get("KERNEL_TRACE", "0"))) and _ensure_ntff_hook()
    try:
        res = run_bass_kernel_spmd(
            nc, in_maps, core_ids=list(range(NCORES)),
            trace=trace, trace_cores=list(range(NCORES)) if trace else None)
    except Exception:
        # transient device faults (e.g. NRT_EXEC_UNIT_UNRECOVERABLE) recover
        # on re-dispatch; retry once
        res = run_bass_kernel_spmd(
            nc, in_maps, core_ids=list(range(NCORES)),
            trace=trace, trace_cores=list(range(NCORES)) if trace else None)
    kernel.last_exec_time_ns = res.exec_time_ns
    kernel.last_results = res

    out = np.empty((N_NODES + N_RELS, MEM_DIM), dtype=np.float32)
    for c in range(NCORES):
        lo_n, hi_n = c * NSHARD, min((c + 1) * NSHARD, N_NODES)
        out[lo_n:hi_n] = res.results[c]["out_n"][:hi_n - lo_n].astype(np.float32)
        lo_r, hi_r = c * RSHARD, min((c + 1) * RSHARD, N_RELS)
        out[N_NODES + lo_r:N_NODES + hi_r] = \
            res.results[c]["out_r"][:hi_r - lo_r].astype(np.float32)
    return out


# revision 48
# speedup vs baseline: 1.1327x; 1.0275x over previous
"""Trainium2 Bass kernel for nn_Memory scatter_memory problem.

Reference computation:
    scale = t/(t+1) if t > 1 else 1
    inv   = 1/(t+1)
    entity_memory = entity_memory*scale ;  .at[nodes_ids].add((nodes_emb @ W_node.T + b_node)*inv)
    rel_memory    = rel_memory*scale    ;  .at[rels_ids].add((rels_emb @ W_rel.T + b_rel)*inv)
    out = concat([entity_memory, rel_memory])   # [100500, 512]

Strategy (8 NeuronCores, SPMD single program):
  - Row-shard entity_memory (12544 rows/core) and rel_memory (64 rows/core).
  - HOST routes each event to its owner core (by id range), sorts by local row id,
    pads to a common chunk count. Bias (zero in practice) is folded into the
    memory shards as cnt*inv*b/scale. Memory flows as bf16 (tolerance 2e-2).
  - NODES (~0.65 events/row): fp8e4 DoubleRow projection matmuls (K=256/pass,
    2x PE throughput), Act-engine scaled psum->SBUF copy, then scatter-add via
    one-hot f32r matmuls into per-row-group PSUM tiles.
  - RELS (~131 events/row; fp8 would lose sqrt(131)x precision): aggregate-first.
    One-hot fp16 matmuls (lhsT=emb k-slice, rhs=onehot[ev,64]) accumulate
    S.T = sum of embeddings per rel row in PSUM [128k, 8, 64]; project S once
    at the end through fp16 W_rel*inv. Exact up to fp16.
  - Host reassembles the full [100500, 512] f32 output from per-core shards.
"""

import os
import sys
import numpy as np

for _p in ("/root/.axon_site", "/root/.axon_site/_ro/trn_rl_repo",
           "/root/.axon_site/_ro/pypackages", "/opt/trn_rl_repo"):
    if os.path.isdir(_p) and _p not in sys.path:
        sys.path.append(_p)

import ml_dtypes
import concourse.bacc as bacc
import concourse.mybir as mybir
import concourse.tile as tile
from concourse.bass_utils import run_bass_kernel_spmd

F32 = mybir.dt.float32
F32R = mybir.dt.float32r
F16 = mybir.dt.float16
BF16 = mybir.dt.bfloat16
FP8 = mybir.dt.float8e4
NP8 = ml_dtypes.float8_e4m3
NPBF = ml_dtypes.bfloat16
AL = mybir.AluOpType
DR = mybir.MatmulPerfMode.DoubleRow

N_NODES = 100000
N_RELS = 500
MEM_DIM = 512
IN_DIM = 1024
NCORES = 8
NSHARD = 12544          # 98 * 128 rows per core (core 7 ragged, padded)
NGROUPS = NSHARD // 128  # 98
RSHARD = 64             # rel rows per core (core 7 ragged, padded)
KT = IN_DIM // 128      # 8 k-tiles
NJ = KT // 2            # 4 DoubleRow pairs
W_PRESCALE = 96.0       # host folds inv*this into W_node so fp8 stays normal
PAD_ID = 1.0e6

_module_cache = {}


def _id_groups(last_pair):
    """Groups whose merge runs via the PE+Act path (host pre-scales their mem
    rows by `scale`). Empty: the identity seed stalled the in-order PE queue
    on the mem DMA and measured slower than the pure-DVE merge path."""
    return set()


def _ensure_ntff_hook():
    """Register the axon NTFF profile hook (missing antenv.axon_hooks shim)."""
    import types
    try:
        from antenv.axon_hooks import get_axon_ntff_profile_hook
        return get_axon_ntff_profile_hook() is not None
    except ImportError:
        pass
    try:
        import antenv
        from trn_agent_boot.trn_boot import _ntff_profile_via_ctypes
        import concourse.bass_utils as bu
        mod = types.ModuleType("antenv.axon_hooks")
        state = {"h": None}
        mod.set_axon_ntff_profile_hook = lambda h: state.__setitem__("h", h)
        mod.get_axon_ntff_profile_hook = lambda: state["h"]
        sys.modules["antenv.axon_hooks"] = mod
        antenv.axon_hooks = mod
        h = _ntff_profile_via_ctypes("/opt/axon/libaxon_pjrt.so")
        mod.set_axon_ntff_profile_hook(h)
        bu.upload_artifacts = lambda tmpdir: f"local:{tmpdir}"
        return h is not None
    except Exception:
        return False


def _build_module(NCn, NCr, spans_n):
    """Build the SPMD Bacc module.

    NCn/NCr: number of 128-event chunks for nodes/rels.
    spans_n: list over ec of sorted group lists (union over cores).
    """
    nc = bacc.Bacc(None, target_bir_lowering=False)

    NPn = (NCn + 1) // 2
    # nodes per chunk: [p=128 (k%128), pair j=4, i=2, event=128] fp8
    emb_n = nc.dram_tensor("emb_n", [NPn, 128, 2 * IN_DIM], FP8, kind="ExternalInput")
    # rels per chunk: [p=128 (event), k=1024] fp16
    emb_r = nc.dram_tensor("emb_r", [NCr, 128, IN_DIM], F16, kind="ExternalInput")
    ids_n = nc.dram_tensor("ids_n", [128, NCn], F32, kind="ExternalInput")
    ids_r = nc.dram_tensor("ids_r", [128, NCr], F32, kind="ExternalInput")
    w_n = nc.dram_tensor("w_n", [128, NJ * 2 * MEM_DIM], FP8, kind="ExternalInput")
    w_r = nc.dram_tensor("w_r", [128, KT * MEM_DIM], F16, kind="ExternalInput")
    s_col = nc.dram_tensor("s_col", [128, 1], F32, kind="ExternalInput")
    iota_in = nc.dram_tensor("iota_in", [128, 128], F32, kind="ExternalInput")
    iota16_in = nc.dram_tensor("iota16_in", [128, RSHARD], F16, kind="ExternalInput")
    ident8_in = nc.dram_tensor("ident8_in", [128, 128], FP8, kind="ExternalInput")
    mem = nc.dram_tensor("mem", [NSHARD, MEM_DIM], FP8, kind="ExternalInput")
    rmem = nc.dram_tensor("rmem", [RSHARD, MEM_DIM], FP8, kind="ExternalInput")
    out_n = nc.dram_tensor("out_n", [NSHARD, MEM_DIM], BF16, kind="ExternalOutput")
    out_r = nc.dram_tensor("out_r", [RSHARD, MEM_DIM], BF16, kind="ExternalOutput")

    # scatter runs per chunk-PAIR (fp8 DoubleRow: K=256 = 2 event chunks)
    NPAIR = (NCn + 1) // 2
    pair_chunks = [[c for c in (2 * a, 2 * a + 1) if c < NCn]
                   for a in range(NPAIR)]
    spans_pair = [sorted(set().union(*[spans_n[c] for c in pcs]))
                  for pcs in pair_chunks]
    touch_in_pair = {}   # (a, g) -> list of pair-local chunk slots (0/1)
    for a, pcs in enumerate(pair_chunks):
        for ec in pcs:
            for g in spans_n[ec]:
                touch_in_pair.setdefault((a, g), []).append(ec % 2)
    last_pair = {}
    for a, gs in enumerate(spans_pair):
        for g in gs:
            last_pair[g] = a
    merge_after = [[] for _ in range(NPAIR)]
    for g, a in last_pair.items():
        merge_after[a].append(g)
    untouched = [g for g in range(NGROUPS) if g not in last_pair]

    # PSUM budget: proj double-buffer + open scatter groups + rel agg/proj banks
    maxopen = 0
    open_now = set()
    for a, gs in enumerate(spans_pair):
        open_now.update(gs)
        maxopen = max(maxopen, len(open_now))
        for g in merge_after[a]:
            open_now.discard(g)
    pu_bufs = 2 if maxopen <= 5 else 1
    pg_bufs = min(max(maxopen, 1), 8 - pu_bufs - 1)

    # ~40% of merges take the PE+Act path (psum seeded with identity@mem, so
    # the merge is a plain Act copy) to unload the saturated DVE
    id_groups = _id_groups(last_pair)

    with tile.TileContext(nc) as tc:
        with tc.tile_pool(name="const", bufs=1) as cpool, \
             tc.tile_pool(name="stage", bufs=6) as spool, \
             tc.tile_pool(name="rstage", bufs=8) as rspool, \
             tc.tile_pool(name="work", bufs=14) as wpool, \
             tc.tile_pool(name="oh", bufs=16) as ohpool, \
             tc.tile_pool(name="updp", bufs=8) as updpool, \
             tc.tile_pool(name="pu", bufs=pu_bufs, space="PSUM") as pupool, \
             tc.tile_pool(name="pg", bufs=pg_bufs, space="PSUM") as pgpool, \
             tc.tile_pool(name="pr", bufs=1, space="PSUM") as prpool:

            # ---- constants (W first: PE-critical path) ----
            t_wn = cpool.tile([128, NJ, 2, MEM_DIM], FP8, tag="wn")
            nc.sync.dma_start(t_wn[:], w_n.ap().rearrange(
                "p (j i n) -> p j i n", j=NJ, i=2))
            t_wr = cpool.tile([128, KT, MEM_DIM], F16, tag="wr")
            nc.sync.dma_start(t_wr[:], w_r.ap().rearrange("p (k n) -> p k n", k=KT))
            t_iota = cpool.tile([128, 128], F32, tag="iota")
            nc.scalar.dma_start(t_iota[:], iota_in[:])
            t_iota16 = cpool.tile([128, RSHARD], F16, tag="iota16")
            nc.scalar.dma_start(t_iota16[:], iota16_in[:])
            t_ids_n = cpool.tile([128, NCn], F32, tag="idsn")
            nc.scalar.dma_start(t_ids_n[:], ids_n[:])
            t_ids_r = cpool.tile([128, NCr], F32, tag="idsr")
            nc.scalar.dma_start(t_ids_r[:], ids_r[:])
            t_s = cpool.tile([128, 1], F32, tag="scol")
            nc.scalar.dma_start(t_s[:], s_col[:])
            t_id8 = cpool.tile([128, 128], FP8, tag="id8")
            nc.scalar.dma_start(t_id8[:], ident8_in[:])

            def fetch_mem(g):
                t_mem = wpool.tile([128, MEM_DIM], FP8, tag="memst",
                                   name=f"mem_{g}")
                nc.sync.dma_start(t_mem[:], mem[g * 128:(g + 1) * 128, :])
                grp_mem[g] = t_mem

            def _merge_into(g, t_out):
                if g not in grp_mem:
                    fetch_mem(g)
                t_mem = grp_mem.pop(g)
                if g in grp_psum:
                    # merges on DVE: GPSIMD cannot access PSUM
                    nc.vector.scalar_tensor_tensor(
                        t_out, t_mem[:], t_s[:, 0:1], grp_psum[g][:],
                        op0=AL.mult, op1=AL.add)
                    del grp_psum[g]
                else:
                    nc.vector.tensor_scalar_mul(t_out, t_mem[:], t_s[:, 0:1])

            def merge_groups(gs):
                """Merge groups gs; runs of consecutive groups (up to 4)
                share one staging tile and one out-DMA."""
                i = 0
                while i < len(gs):
                    L = 1
                    while (L < 4 and i + L < len(gs)
                           and gs[i + L] == gs[i] + L):
                        L += 1
                    g0 = gs[i]
                    t_out = wpool.tile([128, 4, MEM_DIM], BF16, tag="outsb")
                    for j in range(L):
                        _merge_into(g0 + j, t_out[:, j, :])
                    # out-DMA on the idle gpsimd queue (SWDGE); one DMA
                    # covers the whole run of consecutive groups
                    if L == 1:
                        nc.gpsimd.dma_start(
                            out_n[g0 * 128:(g0 + 1) * 128, :], t_out[:, 0, :])
                    else:
                        nc.gpsimd.dma_start(
                            out_n[g0 * 128:(g0 + L) * 128, :]
                            .rearrange("(t p) f -> p t f", t=L),
                            t_out[:, :L, :])
                    i += L

            grp_psum = {}
            grp_mem = {}
            oh_pair = {}
            upd_pair = {}
            pair_n = {}
            upd_scale = 1.0 / W_PRESCALE
            PFP = 3   # node emb prefetch distance (pairs of chunks)
            PFR = 5   # rel emb prefetch distance (chunks)

            def issue_emb_n(pi):
                if pi * 2 < NCn and pi not in pair_n:
                    t_pp = spool.tile([128, 2, NJ, 2, 128], FP8, tag="er",
                                      name=f"en_{pi}")
                    nc.sync.dma_start(
                        t_pp[:], emb_n[pi].rearrange(
                            "p (c j i e) -> p c j i e", c=2, j=NJ, i=2))
                    pair_n[pi] = t_pp

            def node_proj(ec):
                issue_emb_n(ec // 2 + PFP)
                a, half = ec // 2, ec % 2
                t_en = pair_n[a][:, half]
                p_u = pupool.tile([128, MEM_DIM], F32, tag="pu", name=f"pun_{ec}")
                for j in range(NJ):
                    nc.tensor.matmul(p_u[:], t_en[:, j], t_wn[:, j],
                                     start=(j == 0), stop=(j == NJ - 1),
                                     perf_mode=DR)
                for g in spans_n[ec]:
                    if (a, g) not in oh_pair:
                        oh_pair[(a, g)] = ohpool.tile(
                            [128, 2, 128], FP8, tag="oh", name=f"oh_{a}_{g}")
                    # one-hot builds stay on DVE: gpsimd's software
                    # tensor_scalar measured ~20x slower than modeled
                    nc.vector.tensor_scalar(
                        oh_pair[(a, g)][:, half], t_iota[:], float(g * 128),
                        t_ids_n[:, ec:ec + 1], op0=AL.add, op1=AL.is_equal)
                if half == 0:
                    upd_pair[a] = updpool.tile([128, 2, MEM_DIM], FP8, tag="upd",
                                               name=f"updn_{a}")
                nc.scalar.mul(upd_pair[a][:, half], p_u[:], upd_scale)

            def node_scatter_pair(a):
                t_updp = upd_pair.pop(a)
                for g in spans_pair[a]:
                    if g not in grp_psum:
                        grp_psum[g] = pgpool.tile([128, MEM_DIM], F32, tag="pg",
                                                  name=f"pg_{g}")
                        fetch_mem(g)  # prefetch the memory rows for the merge
                        first = True
                    else:
                        first = False
                    t_ohp = oh_pair.pop((a, g))
                    stop = last_pair[g] == a
                    slots = touch_in_pair[(a, g)]
                    if len(slots) == 2:
                        nc.tensor.matmul(grp_psum[g][:], t_ohp[:], t_updp[:],
                                         start=first, stop=stop, perf_mode=DR,
                                         skip_group_check=True)
                    else:
                        i = slots[0]
                        nc.tensor.matmul(grp_psum[g][:], t_ohp[:, i], t_updp[:, i],
                                         start=first, stop=stop,
                                         skip_group_check=True)
                merge_groups(sorted(merge_after[a]))

            rel_er = {}

            def issue_emb_r(ec):
                if ec < NCr and ec not in rel_er:
                    t_er = rspool.tile([128, KT, 128], F16, tag="err",
                                       name=f"er_{ec}")
                    nc.sync.dma_start(
                        t_er[:], emb_r[ec].rearrange("p (k e) -> p k e", k=KT))
                    rel_er[ec] = t_er

            def rel_agg(ec):
                """Accumulate S.T[k, rel_row] += emb[ev, k].T @ onehot[ev, row]."""
                issue_emb_r(ec + PFR)
                t_er = rel_er.pop(ec)
                t_oh = ohpool.tile([128, RSHARD], F16, tag="ohr", name=f"ohr_{ec}")
                nc.vector.tensor_scalar(
                    t_oh[:], t_iota16[:], 0.0, t_ids_r[:, ec:ec + 1],
                    op0=AL.add, op1=AL.is_equal)
                # start=False always: a per-k-slice start would zero the WHOLE
                # tile, wiping sibling slices; the tile is memset once instead.
                for k in range(KT):
                    nc.tensor.matmul(p_aggT[:, k, :], t_er[:, k, :], t_oh[:],
                                     start=False, stop=(ec == NCr - 1),
                                     skip_group_check=True)

            # rel aggregation PSUM: S.T layout [128 (k%128), kt=8, 64 rel rows]
            # (shares its bank with p_rel below — lifetimes are sequential)
            p_aggT = prpool.tile([128, KT, RSHARD], F32, tag="pr", name="pagg")
            nc.vector.memset(p_aggT[:], 0.0)

            # prime the embedding prefetch pipelines
            for pi in range(PFP):
                issue_emb_n(pi)
            for ci in range(PFR):
                issue_emb_r(ci)

            # software-pipelined emission: scatter runs one chunk-pair behind
            # proj, node/rel interleaved to smooth the engine mix
            steps = []
            for i in range(max(NCn, NCr)):
                if i < NCn:
                    steps.append(("n", i))
                if i < NCr:
                    steps.append(("r", i))
            LAGP = 1
            for kind, i in steps:
                if kind == "n":
                    node_proj(i)
                    a = i // 2
                    if (i % 2 == 1 or i == NCn - 1) and a >= LAGP:
                        node_scatter_pair(a - LAGP)
                else:
                    rel_agg(i)
            for a in range(max(NPAIR - LAGP, 0), NPAIR):
                node_scatter_pair(a)

            merge_groups(sorted(untouched))

            # ---- rel projection: S.T -> SBUF fp16, then 8 matmuls ----
            t_sT = wpool.tile([128, KT, RSHARD], F16, tag="sT")
            nc.scalar.copy(t_sT[:], p_aggT[:])
            p_rel = prpool.tile([128, MEM_DIM], F32, tag="pr", name="prel")
            for k in range(KT):
                nc.tensor.matmul(p_rel[:RSHARD, :], t_sT[:, k, :], t_wr[:, k, :],
                                 start=(k == 0), stop=(k == KT - 1))

            # ---- rel merge ----
            t_rmem = wpool.tile([128, MEM_DIM], FP8, tag="memst")
            nc.sync.dma_start(t_rmem[:RSHARD, :], rmem[:])
            t_rout = wpool.tile([128, MEM_DIM], BF16, tag="outsb")
            nc.vector.scalar_tensor_tensor(
                t_rout[:RSHARD, :], t_rmem[:RSHARD, :], t_s[:RSHARD, 0:1],
                p_rel[:RSHARD, :], op0=AL.mult, op1=AL.add)
            nc.gpsimd.dma_start(out_r[:], t_rout[:RSHARD, :])

    nc.finalize()
    return nc


def _route(ids, n_rows_per_core):
    """Route events to owner cores; sort by local id.

    Returns (perm[core] event indices sorted by local id, NC common chunk count).
    """
    owner = np.minimum(ids // n_rows_per_core, NCORES - 1)
    perms = []
    for c in range(NCORES):
        ev = np.nonzero(owner == c)[0]
        loc = ids[ev] - c * n_rows_per_core
        order = np.argsort(loc, kind="stable")
        perms.append(ev[order])
    nmax = max(len(p) for p in perms)
    NC = (nmax + 127) // 128
    return perms, max(NC, 1)


def _pack_emb_nodes(embT_q, perm, NC):
    """embT_q [IN_DIM, B] fp8 -> [NP, 128, 2*IN_DIM] DoubleRow pair layout.

    Per chunk, partition p holds [j, i, e] with k = (2j+i)*128 + p.
    """
    n = len(perm)
    C = NC * 128
    g = np.zeros((IN_DIM, C), dtype=embT_q.dtype)
    g[:, :n] = embT_q[:, perm]
    # [ (j i p), (c e) ] -> [c, p, j, i, e]
    g = g.reshape(NJ, 2, 128, NC, 128).transpose(3, 2, 0, 1, 4)
    g = g.reshape(NC, 128, IN_DIM)
    NP = (NC + 1) // 2
    if NP * 2 != NC:
        g = np.concatenate([g, np.zeros((1, 128, IN_DIM), g.dtype)], axis=0)
    g = g.reshape(NP, 2, 128, IN_DIM).transpose(0, 2, 1, 3).reshape(NP, 128, 2 * IN_DIM)
    return np.ascontiguousarray(g)


def _pack_emb_rels(emb_q, perm, NC):
    """emb_q [B, IN_DIM] fp16 -> [NC, 128, IN_DIM], partition = event."""
    n = len(perm)
    C = NC * 128
    g = np.zeros((C, IN_DIM), dtype=emb_q.dtype)
    g[:n] = emb_q[perm]
    return np.ascontiguousarray(g.reshape(NC, 128, IN_DIM))


def _pack_w_nodes(W, inv):
    """W [MEM_DIM, IN_DIM] -> fp8(W.T * inv * W_PRESCALE) in [128, j, i, m]."""
    wq = (W.T * np.float32(inv * W_PRESCALE)).astype(NP8)  # [IN_DIM, MEM_DIM]
    wq = wq.reshape(NJ, 2, 128, MEM_DIM).transpose(2, 0, 1, 3)
    return np.ascontiguousarray(wq.reshape(128, NJ * 2 * MEM_DIM))


def _pack_w_rels(W, inv):
    """W [MEM_DIM, IN_DIM] -> fp16(W.T * inv) in [128, k, m]."""
    wq = (W.T * np.float32(inv)).astype(np.float16)
    wq = wq.reshape(KT, 128, MEM_DIM).transpose(1, 0, 2)
    return np.ascontiguousarray(wq.reshape(128, KT * MEM_DIM))


def _pack_ids(local_ids, NC):
    n = len(local_ids)
    C = NC * 128
    out = np.full(C, PAD_ID, dtype=np.float32)
    out[:n] = local_ids.astype(np.float32)
    return np.ascontiguousarray(out.reshape(NC, 128).T)  # [128, NC]


def _spans(local_sorted_per_core, NC):
    spans = [set() for _ in range(NC)]
    for loc in local_sorted_per_core:
        for ec in range(NC):
            seg = loc[ec * 128:(ec + 1) * 128]
            if len(seg) == 0:
                continue
            for g in range(int(seg[0]) // 128, int(seg[-1]) // 128 + 1):
                spans[ec].add(g)
    return [sorted(s) for s in spans]


def kernel(nodes_embeddings, rels_embeddings, nodes_ids, rels_ids,
           entity_memory, rel_memory, W_node, b_node, W_rel, b_rel, time):
    nodes_embeddings = np.ascontiguousarray(np.asarray(nodes_embeddings, dtype=np.float32))
    rels_embeddings = np.ascontiguousarray(np.asarray(rels_embeddings, dtype=np.float32))
    nodes_ids = np.asarray(nodes_ids).astype(np.int64)
    rels_ids = np.asarray(rels_ids).astype(np.int64)
    entity_memory = np.asarray(entity_memory, dtype=np.float32)
    rel_memory = np.asarray(rel_memory, dtype=np.float32)
    W_node = np.asarray(W_node, dtype=np.float32)
    b_node = np.asarray(b_node, dtype=np.float32)
    W_rel = np.asarray(W_rel, dtype=np.float32)
    b_rel = np.asarray(b_rel, dtype=np.float32)
    t = float(np.asarray(time))

    inv = np.float32(1.0 / (t + 1.0))
    scale = np.float32(t / (t + 1.0)) if t > 1 else np.float32(1.0)

    # ---- host routing ----
    perms_n, NCn = _route(nodes_ids, NSHARD)
    perms_r, NCr = _route(rels_ids, RSHARD)

    loc_n = [nodes_ids[p] - c * NSHARD for c, p in enumerate(perms_n)]
    spans_n = _spans(loc_n, NCn)

    # mirror the module's merge-path split (host pre-scales id-group mem rows)
    NPAIR = (NCn + 1) // 2
    last_pair = {}
    for a in range(NPAIR):
        for ec in (2 * a, 2 * a + 1):
            if ec < NCn:
                for g in spans_n[ec]:
                    last_pair[g] = a
    idg = sorted(_id_groups(last_pair))

    key = (NCn, NCr, tuple(tuple(s) for s in spans_n))
    if key not in _module_cache:
        _module_cache[key] = _build_module(NCn, NCr, spans_n)
    nc = _module_cache[key]

    # ---- host packing ----
    embT_n = nodes_embeddings.astype(NP8).T  # [IN_DIM, B] fp8
    emb_r16 = rels_embeddings.astype(np.float16)  # [B, IN_DIM] fp16
    wn = _pack_w_nodes(W_node, inv)
    wr = _pack_w_rels(W_rel, inv)
    s_col = np.full((128, 1), scale, dtype=np.float32)
    iota = np.broadcast_to(np.arange(128, dtype=np.float32), (128, 128)).copy()
    iota16 = np.broadcast_to(np.arange(RSHARD, dtype=np.float16), (128, RSHARD)).copy()
    ident8 = np.eye(128, dtype=np.float32).astype(NP8)

    in_maps = []
    for c in range(NCORES):
        lo_n, hi_n = c * NSHARD, min((c + 1) * NSHARD, N_NODES)
        lo_r, hi_r = c * RSHARD, min((c + 1) * RSHARD, N_RELS)
        mem_shard = np.zeros((NSHARD, MEM_DIM), dtype=np.float32)
        mem_shard[:hi_n - lo_n] = entity_memory[lo_n:hi_n]
        rmem_shard = np.zeros((RSHARD, MEM_DIM), dtype=np.float32)
        rmem_shard[:hi_r - lo_r] = rel_memory[lo_r:hi_r]
        loc_r = rels_ids[perms_r[c]] - c * RSHARD
        # fold bias: device computes out = mem*scale + sum(updates); each event
        # contributes inv*b less than the reference, so pre-add cnt*inv*b/scale.
        if b_node.any():
            cnt = np.bincount(loc_n[c], minlength=NSHARD).astype(np.float32)
            mem_shard += (cnt[:, None] * (inv / scale)) * b_node[None, :]
        if b_rel.any():
            cntr = np.bincount(loc_r, minlength=RSHARD).astype(np.float32)
            rmem_shard += (cntr[:, None] * (inv / scale)) * b_rel[None, :]
        for g in idg:
            mem_shard[g * 128:(g + 1) * 128] *= scale
        in_maps.append(dict(
            emb_n=_pack_emb_nodes(embT_n, perms_n[c], NCn),
            emb_r=_pack_emb_rels(emb_r16, perms_r[c], NCr),
            ids_n=_pack_ids(loc_n[c], NCn),
            ids_r=_pack_ids(loc_r, NCr),
            w_n=wn, w_r=wr, s_col=s_col, iota_in=iota, iota16_in=iota16,
            ident8_in=ident8, mem=mem_shard.astype(NP8),
            rmem=rmem_shard.astype(NP8),
        ))

    trace = bool(int(os.environ.get("KERNEL_TRACE", "0"))) and _ensure_ntff_hook()
    try:
        res = run_bass_kernel_spmd(
            nc, in_maps, core_ids=list(range(NCORES)),
            trace=trace, trace_cores=list(range(NCORES)) if trace else None)
    except Exception:
        # transient device faults (e.g. NRT_EXEC_UNIT_UNRECOVERABLE) recover
        # on re-dispatch; retry once
        res = run_bass_kernel_spmd(
            nc, in_maps, core_ids=list(range(NCORES)),
            trace=trace, trace_cores=list(range(NCORES)) if trace else None)
    kernel.last_exec_time_ns = res.exec_time_ns
    kernel.last_results = res

    out = np.empty((N_NODES + N_RELS, MEM_DIM), dtype=np.float32)
    for c in range(NCORES):
        lo_n, hi_n = c * NSHARD, min((c + 1) * NSHARD, N_NODES)
        out[lo_n:hi_n] = res.results[c]["out_n"][:hi_n - lo_n].astype(np.float32)
        lo_r, hi_r = c * RSHARD, min((c + 1) * RSHARD, N_RELS)
        out[N_NODES + lo_r:N_NODES + hi_r] = \
            res.results[c]["out_r"][:hi_r - lo_r].astype(np.float32)
    return out
